# revision 1
# baseline (speedup 1.0000x reference)
"""Trainium2 Bass kernel for nn_CausePredictor (RGCN + pairwise MLP).

Sharding: data-parallel over the pairwise row index i (dim 1 of the
[B,S,S] output): 8 cores x 25 rows, replicated over B=4.  All per-core
differences are encoded as input DATA (column slices / selection
matrices), so one SPMD program serves all cores.

Math (matching reference.py):
  h   = sum_k Ahat_k.T @ (x[b] @ basis_k) + x[b] @ root + bias
        where Ahat_k[i,j] = sum_r comp[r,k] * A[r,i,j] * inv_deg[r,j]
  u   = h @ W1a   (j-indexed term),  v = h @ W1c  (i-indexed term)
  T   = pe_k @ W1b + pe_v @ W1d                  # [11, 512] table
  h1[b,i,j,:] = u[b,j] + v[b,i] + T[pos(i,j)]
  out = sigmoid(relu(relu(h1) @ W2) @ Wp) * mask

On-chip layout is feature-on-partition: [m(128-chunk), pairs] so both
big GEMMs run without activation transposes.
"""

import sys

sys.path.insert(0, "/opt/trn_rl_repo")

import numpy as np

B, S, D, M, P = 4, 200, 300, 512, 100
NREL, MAXL = 9, 10
NCORES = 8
IPC = S // NCORES  # 25 rows of i per core
NU = IPC // 2 + 1  # 13 units per b: 12x 2-row + 1x 1-row
ROWS = B * NU  # 52 output rows per core
FPC = IPC * S  # 5000 pairs per (b, core)

_prog_cache = {}
NB = B  # batches emitted in stage B (debug knob)
NA = B  # batches emitted in stage A
NPER = 10  # peR blocks per mc
SKIP_DVE = False
SKIP_RELU2 = False


def _rel_adj(s):
    ra = np.arange(s)[None, :] - np.arange(s)[:, None]
    for i in range(s):
        ra[i, i + 1 :] = 1
        num = 1
        for o in range(i - 1, -1, -2):
            ra[i, o] = -num
            if o - 1 >= 0:
                ra[i, o - 1] = -num
            num += 1
        ra[i, :i] = np.maximum(ra[i, :i], -8)  # -(WINDOW+1), WINDOW=7
    return ra


def _pack_k(w, width=None):
    """[K, N] -> [128, ceil(K/128)*N], K chunked onto partitions, zero pad."""
    k, n = w.shape
    nch = (k + 127) // 128
    out = np.zeros((128, nch * n), np.float32)
    for c in range(nch):
        r = min(128, k - c * 128)
        out[:r, c * n : c * n + n] = w[c * 128 : c * 128 + r]
    return out


def _build_program():
    import ml_dtypes  # noqa: F401
    import concourse.tile as tile
    from concourse import bacc, mybir

    f32 = mybir.dt.float32
    bf16 = mybir.dt.bfloat16
    AF = mybir.ActivationFunctionType
    OP = mybir.AluOpType

    nc = bacc.Bacc()

    SC = S + IPC  # 225: concat of all-j columns and the core's i-slice
    dxT = nc.declare_dram_parameter("xT", [D, B * SC], f32, isOutput=False)
    dahat = nc.declare_dram_parameter("ahat", [128, 4 * SC], f32, isOutput=False)
    dbasis = nc.declare_dram_parameter("basis", [128, 1800], f32, isOutput=False)
    droot = nc.declare_dram_parameter("root", [128, 900], f32, isOutput=False)
    dbias = nc.declare_dram_parameter("bias", [128, 3], f32, isOutput=False)
    dw1a = nc.declare_dram_parameter("w1a", [128, 1536], f32, isOutput=False)
    dw1c = nc.declare_dram_parameter("w1c", [128, 1536], f32, isOutput=False)
    dw1b = nc.declare_dram_parameter("w1b", [P, M], f32, isOutput=False)
    dw1d = nc.declare_dram_parameter("w1d", [P, M], f32, isOutput=False)
    dpekT = nc.declare_dram_parameter("pekT", [P, MAXL + 1], f32, isOutput=False)
    dpevT = nc.declare_dram_parameter("pevT", [P, MAXL + 1], f32, isOutput=False)
    dw2 = nc.declare_dram_parameter("w2", [128, 4 * M], bf16, isOutput=False)
    dwp = nc.declare_dram_parameter("wp", [128, 4], bf16, isOutput=False)
    dE = nc.declare_dram_parameter("emat", [MAXL + 1, FPC], bf16, isOutput=False)
    dout = nc.declare_dram_parameter("out", [B, NU * 400], f32, isOutput=True)

    DCW = [128, 128, 44]  # D=300 chunks
    JCW = [128, 72]  # S=200 chunks

    with tile.TileContext(nc) as tc:
        with (
            tc.tile_pool(name="persist", bufs=1) as pp,
            tc.tile_pool(name="work", bufs=3) as pwork,
            tc.tile_pool(name="sigp", bufs=2) as psig,
        ):
            def load(name, shape, dt, src):
                t = pp.tile(shape, dt, tag=name, name=name)
                nc.sync.dma_start(t[:, :], src)
                return t

            xT = [load(f"xT{c}", [DCW[c], B * SC], f32,
                       dxT[c * 128 : c * 128 + DCW[c], :]) for c in range(3)]
            basis = load("basis", [128, 1800], f32, dbasis[:, :])
            root = load("root", [128, 900], f32, droot[:, :])
            bias = load("bias", [128, 3], f32, dbias[:, :])
            w1a = load("w1a", [128, 1536], f32, dw1a[:, :])
            w1c = load("w1c", [128, 1536], f32, dw1c[:, :])
            w1b = load("w1b", [P, M], f32, dw1b[:, :])
            w1d = load("w1d", [P, M], f32, dw1d[:, :])
            pekT = load("pekT", [P, MAXL + 1], f32, dpekT[:, :])
            pevT = load("pevT", [P, MAXL + 1], f32, dpevT[:, :])
            ahat = load("ahat", [128, 4 * SC], f32, dahat[:, :])
            w2 = load("w2", [128, 4 * M], bf16, dw2[:, :])
            wp = load("wp", [128, 4], bf16, dwp[:, :])
            Emat = load("emat", [MAXL + 1, FPC], bf16, dE[:, :])

            TtT = pp.tile([MAXL + 1, M], bf16, tag="TtT", name="TtT")
            peR = [pp.tile([128, FPC], bf16, tag=f"peR{mc}", name=f"peR{mc}") for mc in range(4)]
            hT = [[pp.tile([DCW[ec], SC], f32, tag=f"hT{b}{ec}", name=f"hT{b}{ec}")
                   for ec in range(3)] for b in range(B)]
            uT = [[pp.tile([128, S], bf16, tag=f"uT{b}{mc}", name=f"uT{b}{mc}")
                   for mc in range(4)] for b in range(B)]
            vT = [[pp.tile([128, IPC], f32, tag=f"vT{b}{mc}", name=f"vT{b}{mc}")
                   for mc in range(4)] for b in range(B)]

            # ---------------- stage A: T table + pe rows + h/u/v ----------
            with tc.tile_pool(name="psT", bufs=1, space="PSUM") as psT:
                # T table [11, 512]
                tps = psT.tile([MAXL + 1, M], f32, tag="tps", name="tps")
                nc.tensor.matmul(tps[:, :], pekT[:, :], w1b[:, :], start=True, stop=False)
                nc.tensor.matmul(tps[:, :], pevT[:, :], w1d[:, :], start=False, stop=True)
                nc.scalar.activation(TtT[:, :], tps[:, :], AF.Copy)

            with tc.tile_pool(name="psA", bufs=2, space="PSUM") as psA:
                # pe rows: peR[mc][:, f] = T[pos(f), mc*128:+128]
                for mc in range(4):
                    for blk in range(NPER):
                        eps = psA.tile([128, 500], f32, tag="mps", name="eps")
                        nc.tensor.matmul(
                            eps[:, :],
                            TtT[:, mc * 128 : mc * 128 + 128],
                            Emat[:, blk * 500 : blk * 500 + 500],
                            start=True, stop=True,
                        )
                        if blk % 2 == 0:
                            nc.scalar.activation(
                                peR[mc][:, blk * 500 : blk * 500 + 500], eps[:, :], AF.Copy)
                        else:
                            nc.vector.tensor_copy(
                                peR[mc][:, blk * 500 : blk * 500 + 500], eps[:, :])

                t1 = [[[pp.tile([JCW[jc], D], f32, tag=f"t1_{b}{k}{jc}",
                                name=f"t1_{b}{k}{jc}")
                        for jc in range(2)] for k in range(2)] for b in range(B)]
                # phase 1: all t1 groups (b-independent -> PE never stalls)
                for b in range(NA):
                    for k in range(2):
                        for jc in range(2):
                            t1ps = psA.tile([JCW[jc], D], f32, tag="mps", name="t1ps")
                            for dc in range(3):
                                nc.tensor.matmul(
                                    t1ps[:, :],
                                    xT[dc][:, b * SC + jc * 128 : b * SC + jc * 128 + JCW[jc]],
                                    basis[0 : DCW[dc], (k * 3 + dc) * D : (k * 3 + dc) * D + D],
                                    start=(dc == 0), stop=(dc == 2),
                                )
                            nc.vector.tensor_copy(t1[b][k][jc][:, :], t1ps[:, :])
                # phase 2: all h groups
                for b in range(NA):
                    for ec in range(3):
                        hps = psA.tile([DCW[ec], SC], f32, tag="hps", name="hps")
                        first = True
                        for k in range(2):
                            for jc in range(2):
                                nc.tensor.matmul(
                                    hps[:, :],
                                    t1[b][k][jc][:, ec * 128 : ec * 128 + DCW[ec]],
                                    ahat[0 : JCW[jc], (k * 2 + jc) * SC : (k * 2 + jc + 1) * SC],
                                    start=first, stop=False)
                                first = False
                        for dc in range(3):
                            nc.tensor.matmul(
                                hps[:, :],
                                root[0 : DCW[dc], dc * D + ec * 128 : dc * D + ec * 128 + DCW[ec]],
                                xT[dc][:, b * SC : (b + 1) * SC],
                                start=False, stop=(dc == 2))
                        nc.scalar.activation(hT[b][ec][:, :], hps[:, :], AF.Identity,
                                             bias=bias[0 : DCW[ec], ec : ec + 1])
                # phase 3: all u/v groups
                for b in range(NA):
                    for mc in range(4):
                        ups = psA.tile([128, SC], f32, tag="uvps", name="ups")
                        for ec in range(3):
                            nc.tensor.matmul(
                                ups[:, 0:S],
                                w1a[0 : DCW[ec], ec * M + mc * 128 : ec * M + mc * 128 + 128],
                                hT[b][ec][:, 0:S], start=(ec == 0), stop=False)
                        for ec in range(3):
                            nc.tensor.matmul(
                                ups[:, S:SC],
                                w1c[0 : DCW[ec], ec * M + mc * 128 : ec * M + mc * 128 + 128],
                                hT[b][ec][:, S:SC], start=(ec == 0), stop=(ec == 2))
                        nc.vector.tensor_copy(uT[b][mc][:, :], ups[:, 0:S])
                        nc.vector.tensor_copy(vT[b][mc][:, :], ups[:, S:SC])

            # ---------------- stage B: the pairwise MLP ------------------
            with (
                tc.tile_pool(name="ps2", bufs=5, space="PSUM") as ps2,
                tc.tile_pool(name="ps3", bufs=3, space="PSUM") as ps3,
            ):
                def emit_g3(prev):
                    p_b, p_sigb, p_u, p_ncols, p_rh2 = prev
                    g3 = ps3.tile([1, 400], f32, tag="g3", name="g3")
                    for n in range(4):
                        nc.tensor.matmul(
                            g3[:, :p_ncols], wp[:, n : n + 1], p_rh2[n][:, :p_ncols],
                            start=(n == 0), stop=(n == 3))
                    nc.scalar.activation(
                        p_sigb[0:1, p_u * 400 : p_u * 400 + p_ncols],
                        g3[:, :p_ncols], AF.Sigmoid)
                    if p_u == NU - 1:
                        nc.sync.dma_start(dout[p_b : p_b + 1, :], p_sigb[0:1, :])

                prev = None
                for b in range(NB):
                    sigb = psig.tile([1, NU * 400], f32, tag="sigb", name="sigb")
                    for u in range(NU):
                        nil = 2 if u < NU - 1 else 1
                        ncols = nil * S
                        rh1 = [pwork.tile([128, 400], bf16, tag=f"rh1_{mc}", name=f"rh1_{mc}")
                               for mc in range(4)]
                        for mc in range(4 if not SKIP_DVE else 0):
                            for h in range(nil):
                                nc.vector.tensor_add(
                                    rh1[mc][:, h * S : h * S + S],
                                    uT[b][mc][:, :],
                                    peR[mc][:, u * 400 + h * S : u * 400 + h * S + S])
                                nc.vector.tensor_scalar(
                                    out=rh1[mc][:, h * S : h * S + S],
                                    in0=rh1[mc][:, h * S : h * S + S],
                                    scalar1=vT[b][mc][:, 2 * u + h : 2 * u + h + 1],
                                    scalar2=0.0,
                                    op0=OP.add, op1=OP.max)
                        # GEMM2 + relu2
                        rh2 = [pwork.tile([128, 400], bf16, tag=f"rh2_{n}", name=f"rh2_{n}")
                               for n in range(4)]
                        for n in range(4):
                            ops = ps2.tile([128, 400], f32, tag="ops", name="ops")
                            for mc in range(4):
                                nc.tensor.matmul(
                                    ops[:, :ncols],
                                    w2[:, mc * M + n * 128 : mc * M + n * 128 + 128],
                                    rh1[mc][:, :ncols],
                                    start=(mc == 0), stop=(mc == 3))
                            (None if SKIP_RELU2 else nc.scalar.activation(rh2[n][:, :ncols], ops[:, :ncols], AF.Relu))
                        # GEMM3 + sigmoid of the PREVIOUS unit (hides relu2 latency)
                        if prev is not None:
                            emit_g3(prev)
                        prev = (b, sigb, u, ncols, rh2)
                emit_g3(prev)

    nc.compile()
    return nc


def _host_prep(x, pe_k, pe_v, comp, basis, root, rgcn_bias, W1):
    import ml_dtypes

    ra = _rel_adj(S) % NREL
    onehot = (ra[None, :, :] == np.arange(NREL)[:, None, None]).astype(np.float64)
    deg = onehot.sum(1)
    inv = np.where(deg > 0, 1.0 / np.maximum(deg, 1.0), 0.0)
    anorm = onehot * inv[:, None, :]
    ahat_full = np.einsum("rk,rij->kij", np.asarray(comp, np.float64), anorm)
    ahat_full = ahat_full.astype(np.float32)  # [2, S, S]
    pos = np.clip(np.arange(S)[:, None] - np.arange(S)[None, :] + 1, 0, MAXL)

    x = np.asarray(x, np.float32)
    W1 = np.asarray(W1, np.float32)
    W1a, W1b = W1[:D], W1[D : D + P]
    W1c, W1d = W1[D + P : 2 * D + P], W1[2 * D + P :]

    com = {
        "basis": np.concatenate(
            [_pack_k(np.asarray(basis[k], np.float32)) for k in range(2)], axis=1),
        "root": _pack_k(np.asarray(root, np.float32)),
        "w1a": _pack_k(W1a),
        "w1c": _pack_k(W1c),
        "w1b": np.ascontiguousarray(W1b),
        "w1d": np.ascontiguousarray(W1d),
        "pekT": np.ascontiguousarray(np.asarray(pe_k, np.float32).T),
        "pevT": np.ascontiguousarray(np.asarray(pe_v, np.float32).T),
        "w2": np.ascontiguousarray(
            np.asarray(W2_GLOBAL, np.float32).reshape(4, 128, M)
            .transpose(1, 0, 2).reshape(128, 4 * M)).astype(ml_dtypes.bfloat16),
        "wp": np.ascontiguousarray(np.asarray(WP_GLOBAL, np.float32)[:, 0]
                                   .reshape(4, 128).T).astype(ml_dtypes.bfloat16),
    }
    bias_p = np.zeros((128, 3), np.float32)
    rb = np.asarray(rgcn_bias, np.float32)
    for c in range(3):
        r = min(128, D - c * 128)
        bias_p[:r, c] = rb[c * 128 : c * 128 + r]
    com["bias"] = bias_p


    SC = S + IPC
    xt_all = x.transpose(2, 0, 1)  # [D, B, S]
    per_core = []
    for c in range(NCORES):
        i0 = c * IPC
        m = dict(com)
        xtc = np.empty((D, B * SC), np.float32)
        for b in range(B):
            xtc[:, b * SC : b * SC + S] = xt_all[:, b, :]
            xtc[:, b * SC + S : (b + 1) * SC] = xt_all[:, b, i0 : i0 + IPC]
        m["xT"] = xtc
        ah = np.zeros((128, 4 * SC), np.float32)
        for k in range(2):
            for jc in range(2):
                r = 128 if jc == 0 else 72
                base = (k * 2 + jc) * SC
                ah[:r, base : base + S] = ahat_full[k, jc * 128 : jc * 128 + r, :]
                ah[:r, base + S : base + SC] = ahat_full[k, jc * 128 : jc * 128 + r, i0 : i0 + IPC]
        m["ahat"] = ah
        E = np.zeros((MAXL + 1, FPC), np.float32)
        E[pos[i0 : i0 + IPC, :].reshape(-1), np.arange(FPC)] = 1.0
        m["emat"] = E.astype(ml_dtypes.bfloat16)
        per_core.append(m)
    return per_core


W2_GLOBAL = None
WP_GLOBAL = None


def kernel(x, mask, pe_k, pe_v, comp, basis, root, rgcn_bias, W1, W2, Wp,
           _want_results=False, _trace=False):
    global W2_GLOBAL, WP_GLOBAL
    W2_GLOBAL, WP_GLOBAL = W2, Wp

    from concourse.bass_utils import run_bass_kernel_spmd

    if "nc" not in _prog_cache:
        _prog_cache["nc"] = _build_program()
    nc = _prog_cache["nc"]

    in_maps = _host_prep(x, pe_k, pe_v, comp, basis, root, rgcn_bias, W1)
    res = run_bass_kernel_spmd(nc, in_maps, core_ids=list(range(NCORES)),
                               trace=_trace)

    out = np.zeros((B, S, S), np.float32)
    for c in range(NCORES):
        i0 = c * IPC
        rows = res.results[c]["out"].reshape(B, NU, 400)
        for b in range(B):
            for u in range(NU - 1):
                out[b, i0 + 2 * u, :] = rows[b, u, :S]
                out[b, i0 + 2 * u + 1, :] = rows[b, u, S:]
            out[b, i0 + IPC - 1, :] = rows[b, NU - 1, :S]
    out *= np.asarray(mask, np.float32)
    if _want_results:
        return out, res
    return out



# revision 2
# speedup vs baseline: 1.4424x; 1.4424x over previous
"""Trainium2 Bass kernel for nn_CausePredictor (RGCN + pairwise MLP).

Sharding: data-parallel over the pairwise row index i (dim 1 of the
[B,S,S] output): 8 cores x 25 rows, replicated over B=4.  All per-core
differences are encoded as input DATA (column slices / selection
matrices), so one SPMD program serves all cores.

Math (matching reference.py):
  h   = sum_k Ahat_k.T @ (x[b] @ basis_k) + x[b] @ root + bias
        where Ahat_k[i,j] = sum_r comp[r,k] * A[r,i,j] * inv_deg[r,j]
  u   = h @ W1a   (j-indexed term),  v = h @ W1c  (i-indexed term)
  T   = pe_k @ W1b + pe_v @ W1d      # [11, 512] table (HOST precomputed)
  h1[b,i,j,:] = u[b,j] + v[b,i] + T[pos(i,j)]
  out = sigmoid(Wp . relu(relu(h1) @ W2)) * mask

On-chip layout is feature-on-partition: [m(128-chunk), pairs] so both
big GEMMs run without activation transposes.  GEMM3 (the Wp dot) runs
with h2 as the STATIONARY operand and wp moving: out is [pairs<=128, 1]
so each matmul costs ~1 PE row instead of 400.  Stage A runs in bf16
(fp32 matmuls are 4x slower on the PE).
"""

import sys

sys.path.insert(0, "/opt/trn_rl_repo")

import numpy as np

B, S, D, M, P = 4, 200, 300, 512, 100
NREL, MAXL = 9, 10
NCORES = 8
IPC = S // NCORES  # 25 rows of i per core
NU = IPC // 2 + 1  # 13 units per b: 12x 2-row + 1x 1-row
FPC = IPC * S  # 5000 pairs per (b, core)
NCOL = 50  # output columns per b: each col = 100 pairs

_prog_cache = {}
NPER = 10  # peR blocks per mc


def _rel_adj(s):
    ra = np.arange(s)[None, :] - np.arange(s)[:, None]
    for i in range(s):
        ra[i, i + 1 :] = 1
        num = 1
        for o in range(i - 1, -1, -2):
            ra[i, o] = -num
            if o - 1 >= 0:
                ra[i, o - 1] = -num
            num += 1
        ra[i, :i] = np.maximum(ra[i, :i], -8)  # -(WINDOW+1), WINDOW=7
    return ra


def _pack_k(w, width=None):
    """[K, N] -> [128, ceil(K/128)*N], K chunked onto partitions, zero pad."""
    k, n = w.shape
    nch = (k + 127) // 128
    out = np.zeros((128, nch * n), np.float32)
    for c in range(nch):
        r = min(128, k - c * 128)
        out[:r, c * n : c * n + n] = w[c * 128 : c * 128 + r]
    return out


def _build_program():
    import ml_dtypes  # noqa: F401
    import concourse.tile as tile
    from concourse import bacc, mybir

    f32 = mybir.dt.float32
    bf16 = mybir.dt.bfloat16
    AF = mybir.ActivationFunctionType
    OP = mybir.AluOpType

    nc = bacc.Bacc()

    SC = S + IPC  # 225: concat of all-j columns and the core's i-slice
    dxT = nc.declare_dram_parameter("xT", [D, B * SC], bf16, isOutput=False)
    dahat = nc.declare_dram_parameter("ahat", [128, 4 * SC], bf16, isOutput=False)
    dbasis = nc.declare_dram_parameter("basis", [128, 1800], bf16, isOutput=False)
    droot = nc.declare_dram_parameter("root", [128, 900], bf16, isOutput=False)
    dbias = nc.declare_dram_parameter("bias", [128, 3], f32, isOutput=False)
    dw1a = nc.declare_dram_parameter("w1a", [128, 1536], bf16, isOutput=False)
    dw1c = nc.declare_dram_parameter("w1c", [128, 1536], bf16, isOutput=False)
    dttab = nc.declare_dram_parameter("ttab", [MAXL + 1, M], bf16, isOutput=False)
    dw2 = nc.declare_dram_parameter("w2", [128, 4 * M], bf16, isOutput=False)
    dwp = nc.declare_dram_parameter("wp", [128, 4], bf16, isOutput=False)
    dE = nc.declare_dram_parameter("emat", [MAXL + 1, FPC], bf16, isOutput=False)
    dout = nc.declare_dram_parameter("out", [B * 100, NCOL], f32, isOutput=True)

    DCW = [128, 128, 44]  # D=300 chunks
    JCW = [128, 72]  # S=200 chunks

    with tile.TileContext(nc) as tc:
        with (
            tc.tile_pool(name="persist", bufs=1) as pp,
            tc.tile_pool(name="work", bufs=3) as pwork,
            tc.tile_pool(name="sigp", bufs=2) as psig,
        ):
            def load(name, shape, dt, src):
                t = pp.tile(shape, dt, tag=name, name=name)
                nc.sync.dma_start(t[:, :], src)
                return t

            xT = [load(f"xT{c}", [DCW[c], B * SC], bf16,
                       dxT[c * 128 : c * 128 + DCW[c], :]) for c in range(3)]
            basis = load("basis", [128, 1800], bf16, dbasis[:, :])
            root = load("root", [128, 900], bf16, droot[:, :])
            bias = load("bias", [128, 3], f32, dbias[:, :])
            w1a = load("w1a", [128, 1536], bf16, dw1a[:, :])
            w1c = load("w1c", [128, 1536], bf16, dw1c[:, :])
            TtT = load("ttab", [MAXL + 1, M], bf16, dttab[:, :])
            ahat = load("ahat", [128, 4 * SC], bf16, dahat[:, :])
            w2 = load("w2", [128, 4 * M], bf16, dw2[:, :])
            wp = load("wp", [128, 4], bf16, dwp[:, :])
            Emat = load("emat", [MAXL + 1, FPC], bf16, dE[:, :])

            peR = [pp.tile([128, FPC], bf16, tag=f"peR{mc}", name=f"peR{mc}") for mc in range(4)]
            hT = [[pp.tile([DCW[ec], SC], bf16, tag=f"hT{b}{ec}", name=f"hT{b}{ec}")
                   for ec in range(3)] for b in range(B)]
            uT = [[pp.tile([128, S], bf16, tag=f"uT{b}{mc}", name=f"uT{b}{mc}")
                   for mc in range(4)] for b in range(B)]
            vT = [[pp.tile([128, IPC], f32, tag=f"vT{b}{mc}", name=f"vT{b}{mc}")
                   for mc in range(4)] for b in range(B)]

            # ---------------- stage A: pe rows + h/u/v ----------
            with tc.tile_pool(name="psA", bufs=2, space="PSUM") as psA:
                # pe rows: peR[mc][:, f] = T[pos(f), mc*128:+128]
                for mc in range(4):
                    for blk in range(NPER):
                        eps = psA.tile([128, 500], f32, tag="mps", name="eps")
                        nc.tensor.matmul(
                            eps[:, :],
                            TtT[:, mc * 128 : mc * 128 + 128],
                            Emat[:, blk * 500 : blk * 500 + 500],
                            start=True, stop=True,
                        )
                        if blk % 2 == 0:
                            nc.scalar.activation(
                                peR[mc][:, blk * 500 : blk * 500 + 500], eps[:, :], AF.Copy)
                        else:
                            nc.vector.tensor_copy(
                                peR[mc][:, blk * 500 : blk * 500 + 500], eps[:, :])

                t1 = [[[pp.tile([JCW[jc], D], bf16, tag=f"t1_{b}{k}{jc}",
                                name=f"t1_{b}{k}{jc}")
                        for jc in range(2)] for k in range(2)] for b in range(B)]
                # phase 1: all t1 groups (b-independent -> PE never stalls)
                for b in range(B):
                    for k in range(2):
                        for jc in range(2):
                            t1ps = psA.tile([JCW[jc], D], f32, tag="mps", name="t1ps")
                            for dc in range(3):
                                nc.tensor.matmul(
                                    t1ps[:, :],
                                    xT[dc][:, b * SC + jc * 128 : b * SC + jc * 128 + JCW[jc]],
                                    basis[0 : DCW[dc], (k * 3 + dc) * D : (k * 3 + dc) * D + D],
                                    start=(dc == 0), stop=(dc == 2),
                                )
                            nc.vector.tensor_copy(t1[b][k][jc][:, :], t1ps[:, :])
                # phase 2: all h groups
                for b in range(B):
                    for ec in range(3):
                        hps = psA.tile([DCW[ec], SC], f32, tag="hps", name="hps")
                        first = True
                        for k in range(2):
                            for jc in range(2):
                                nc.tensor.matmul(
                                    hps[:, :],
                                    t1[b][k][jc][:, ec * 128 : ec * 128 + DCW[ec]],
                                    ahat[0 : JCW[jc], (k * 2 + jc) * SC : (k * 2 + jc + 1) * SC],
                                    start=first, stop=False)
                                first = False
                        for dc in range(3):
                            nc.tensor.matmul(
                                hps[:, :],
                                root[0 : DCW[dc], dc * D + ec * 128 : dc * D + ec * 128 + DCW[ec]],
                                xT[dc][:, b * SC : (b + 1) * SC],
                                start=False, stop=(dc == 2))
                        nc.scalar.activation(hT[b][ec][:, :], hps[:, :], AF.Identity,
                                             bias=bias[0 : DCW[ec], ec : ec + 1])
                # phase 3: all u/v groups
                for b in range(B):
                    for mc in range(4):
                        ups = psA.tile([128, SC], f32, tag="uvps", name="ups")
                        for ec in range(3):
                            nc.tensor.matmul(
                                ups[:, 0:S],
                                w1a[0 : DCW[ec], ec * M + mc * 128 : ec * M + mc * 128 + 128],
                                hT[b][ec][:, 0:S], start=(ec == 0), stop=False)
                        for ec in range(3):
                            nc.tensor.matmul(
                                ups[:, S:SC],
                                w1c[0 : DCW[ec], ec * M + mc * 128 : ec * M + mc * 128 + 128],
                                hT[b][ec][:, S:SC], start=(ec == 0), stop=(ec == 2))
                        nc.vector.tensor_copy(uT[b][mc][:, :], ups[:, 0:S])
                        nc.vector.tensor_copy(vT[b][mc][:, :], ups[:, S:SC])

            # ---------------- stage B: the pairwise MLP ------------------
            with (
                tc.tile_pool(name="ps2", bufs=5, space="PSUM") as ps2,
                tc.tile_pool(name="pp3", bufs=2, space="PSUM") as pp3,
            ):
                def emit_g3(prev):
                    # GEMM3 for the previous unit: h2 chunks stationary,
                    # wp moving -> out [pairs<=100, 1] in the b's pout col.
                    p_u, p_nch, p_pout, p_rh2 = prev
                    for pc in range(p_nch):
                        col = p_u * 4 + pc
                        for mc in range(4):
                            nc.tensor.matmul(
                                p_pout[0:100, col : col + 1],
                                p_rh2[mc][:, pc * 100 : pc * 100 + 100],
                                wp[:, mc : mc + 1],
                                start=(mc == 0), stop=(mc == 3))

                prev = None
                for b in range(B):
                    pout = pp3.tile([128, NCOL], f32, tag="pout", name="pout")
                    for u in range(NU):
                        nil = 2 if u < NU - 1 else 1
                        ncols = nil * S
                        rh1 = [pwork.tile([128, 400], bf16, tag=f"rh1_{mc}", name=f"rh1_{mc}")
                               for mc in range(4)]
                        for mc in range(4):
                            for h in range(nil):
                                nc.vector.tensor_add(
                                    rh1[mc][:, h * S : h * S + S],
                                    uT[b][mc][:, :],
                                    peR[mc][:, u * 400 + h * S : u * 400 + h * S + S])
                                nc.vector.tensor_scalar(
                                    out=rh1[mc][:, h * S : h * S + S],
                                    in0=rh1[mc][:, h * S : h * S + S],
                                    scalar1=vT[b][mc][:, 2 * u + h : 2 * u + h + 1],
                                    scalar2=0.0,
                                    op0=OP.add, op1=OP.max)
                        # GEMM2 + relu2
                        rh2 = [pwork.tile([128, 400], bf16, tag=f"rh2_{n}", name=f"rh2_{n}")
                               for n in range(4)]
                        for n in range(4):
                            ops = ps2.tile([128, 400], f32, tag="ops", name="ops")
                            for mc in range(4):
                                nc.tensor.matmul(
                                    ops[:, :ncols],
                                    w2[:, mc * M + n * 128 : mc * M + n * 128 + 128],
                                    rh1[mc][:, :ncols],
                                    start=(mc == 0), stop=(mc == 3))
                            nc.scalar.activation(rh2[n][:, :ncols], ops[:, :ncols], AF.Relu)
                        # GEMM3 of the PREVIOUS unit (hides relu2 latency)
                        if prev is not None:
                            emit_g3(prev)
                        prev = (u, 4 if nil == 2 else 2, pout, rh2)
                    emit_g3(prev)
                    prev = None
                    sig = psig.tile([128, NCOL], f32, tag="sigb", name="sigb")
                    nc.scalar.activation(sig[0:100, :], pout[0:100, :], AF.Sigmoid)
                    nc.sync.dma_start(dout[b * 100 : b * 100 + 100, :], sig[0:100, :])

    nc.compile()
    return nc


def _host_prep(x, pe_k, pe_v, comp, basis, root, rgcn_bias, W1):
    import ml_dtypes

    bf = ml_dtypes.bfloat16

    ra = _rel_adj(S) % NREL
    onehot = (ra[None, :, :] == np.arange(NREL)[:, None, None]).astype(np.float64)
    deg = onehot.sum(1)
    inv = np.where(deg > 0, 1.0 / np.maximum(deg, 1.0), 0.0)
    anorm = onehot * inv[:, None, :]
    ahat_full = np.einsum("rk,rij->kij", np.asarray(comp, np.float64), anorm)
    ahat_full = ahat_full.astype(np.float32)  # [2, S, S]
    pos = np.clip(np.arange(S)[:, None] - np.arange(S)[None, :] + 1, 0, MAXL)

    x = np.asarray(x, np.float32)
    W1 = np.asarray(W1, np.float32)
    W1a, W1b = W1[:D], W1[D : D + P]
    W1c, W1d = W1[D + P : 2 * D + P], W1[2 * D + P :]
    ttab = (np.asarray(pe_k, np.float64) @ W1b.astype(np.float64)
            + np.asarray(pe_v, np.float64) @ W1d.astype(np.float64)).astype(np.float32)

    com = {
        "basis": np.concatenate(
            [_pack_k(np.asarray(basis[k], np.float32)) for k in range(2)], axis=1
        ).astype(bf),
        "root": _pack_k(np.asarray(root, np.float32)).astype(bf),
        "w1a": _pack_k(W1a).astype(bf),
        "w1c": _pack_k(W1c).astype(bf),
        "ttab": np.ascontiguousarray(ttab).astype(bf),
        "w2": np.ascontiguousarray(
            np.asarray(W2_GLOBAL, np.float32).reshape(4, 128, M)
            .transpose(1, 0, 2).reshape(128, 4 * M)).astype(bf),
        "wp": np.ascontiguousarray(np.asarray(WP_GLOBAL, np.float32)[:, 0]
                                   .reshape(4, 128).T).astype(bf),
    }
    bias_p = np.zeros((128, 3), np.float32)
    rb = np.asarray(rgcn_bias, np.float32)
    for c in range(3):
        r = min(128, D - c * 128)
        bias_p[:r, c] = rb[c * 128 : c * 128 + r]
    com["bias"] = bias_p

    SC = S + IPC
    xt_all = x.transpose(2, 0, 1)  # [D, B, S]
    per_core = []
    for c in range(NCORES):
        i0 = c * IPC
        m = dict(com)
        xtc = np.empty((D, B * SC), np.float32)
        for b in range(B):
            xtc[:, b * SC : b * SC + S] = xt_all[:, b, :]
            xtc[:, b * SC + S : (b + 1) * SC] = xt_all[:, b, i0 : i0 + IPC]
        m["xT"] = xtc.astype(bf)
        ah = np.zeros((128, 4 * SC), np.float32)
        for k in range(2):
            for jc in range(2):
                r = 128 if jc == 0 else 72
                base = (k * 2 + jc) * SC
                ah[:r, base : base + S] = ahat_full[k, jc * 128 : jc * 128 + r, :]
                ah[:r, base + S : base + SC] = ahat_full[k, jc * 128 : jc * 128 + r, i0 : i0 + IPC]
        m["ahat"] = ah.astype(bf)
        E = np.zeros((MAXL + 1, FPC), np.float32)
        E[pos[i0 : i0 + IPC, :].reshape(-1), np.arange(FPC)] = 1.0
        m["emat"] = E.astype(bf)
        per_core.append(m)
    return per_core


W2_GLOBAL = None
WP_GLOBAL = None


def kernel(x, mask, pe_k, pe_v, comp, basis, root, rgcn_bias, W1, W2, Wp,
           _want_results=False, _trace=False):
    global W2_GLOBAL, WP_GLOBAL
    W2_GLOBAL, WP_GLOBAL = W2, Wp

    from concourse.bass_utils import run_bass_kernel_spmd

    if "nc" not in _prog_cache:
        _prog_cache["nc"] = _build_program()
    nc = _prog_cache["nc"]

    in_maps = _host_prep(x, pe_k, pe_v, comp, basis, root, rgcn_bias, W1)
    res = run_bass_kernel_spmd(nc, in_maps, core_ids=list(range(NCORES)),
                               trace=_trace)

    out = np.zeros((B, S, S), np.float32)
    for c in range(NCORES):
        i0 = c * IPC
        arr = np.asarray(res.results[c]["out"], np.float32).reshape(B, 100, NCOL)
        out[:, i0 : i0 + IPC, :] = arr.transpose(0, 2, 1).reshape(B, IPC, S)
    out *= np.asarray(mask, np.float32)
    if _want_results:
        return out, res
    return out


# revision 5
# speedup vs baseline: 1.9018x; 1.3185x over previous
"""Trainium2 Bass kernel for nn_CausePredictor (RGCN + pairwise MLP).

Sharding: data-parallel over the pairwise row index i (dim 1 of the
[B,S,S] output): 8 cores x 25 rows, replicated over B=4.  All per-core
differences are encoded as input DATA (column slices / gathered pe
tables), so one SPMD program serves all cores.

Math (matching reference.py):
  h   = sum_k Ahat_k.T @ (x[b] @ basis_k) + x[b] @ root + bias
  u   = h @ W1a   (j term),  v = h @ W1c  (i term)
  T   = pe_k @ W1b + pe_v @ W1d          # [11, 512], host precomputed
  h1[b,i,j,:] = u[b,j] + v[b,i] + T[pos(i,j)]
  out = sigmoid(Wp . relu(relu(h1) @ W2)) * mask

Performance structure (per TimelineSim cost model):
  - stage A (RGCN + u/v) runs in bf16 (fp32 matmuls are 4x slower).
  - peR = T[pos] pair-expansion is precomputed on HOST and DMA'd in.
  - t = u + peR is ONE broadcast tensor_add per (b, mc) on DVE.
  - rh1 = relu(t + v_i): K-chunks 0,1 quantize to fp8 (on Pool),
    chunks 2,3 stay bf16 (on DVE).
  - GEMM2: fp8 chunks use DoubleRow matmuls (0.5 cyc/row) with W2
    split into hi+lo fp8 pair (weight quantization ~exact); bf16
    chunks use normal matmuls.  3 cyc/row total vs 4 for pure bf16.
  - relu2 PSUM->SBUF copies split 3:1 between Activation and Pool.
  - GEMM3 (the Wp dot) runs with h2 chunks STATIONARY and wp moving:
    out is [pairs<=100, 1], ~1 PE row per matmul instead of 400.
"""

import sys

sys.path.insert(0, "/opt/trn_rl_repo")

import numpy as np

B, S, D, M, P = 4, 200, 300, 512, 100
NREL, MAXL = 9, 10
NCORES = 8
IPC = S // NCORES  # 25 rows of i per core
NU = IPC // 2 + 1  # 13 units per b: 12x 2-row + 1x 1-row
FPC = IPC * S  # 5000 pairs per (b, core)
NCOL = 50  # output columns per b: each col = 100 pairs

_prog_cache = {}


def _rel_adj(s):
    ra = np.arange(s)[None, :] - np.arange(s)[:, None]
    for i in range(s):
        ra[i, i + 1 :] = 1
        num = 1
        for o in range(i - 1, -1, -2):
            ra[i, o] = -num
            if o - 1 >= 0:
                ra[i, o - 1] = -num
            num += 1
        ra[i, :i] = np.maximum(ra[i, :i], -8)  # -(WINDOW+1), WINDOW=7
    return ra


def _pack_k(w, width=None):
    """[K, N] -> [128, ceil(K/128)*N], K chunked onto partitions, zero pad."""
    k, n = w.shape
    nch = (k + 127) // 128
    out = np.zeros((128, nch * n), np.float32)
    for c in range(nch):
        r = min(128, k - c * 128)
        out[:r, c * n : c * n + n] = w[c * 128 : c * 128 + r]
    return out


def _build_program():
    import ml_dtypes  # noqa: F401
    import concourse.tile as tile
    from concourse import bacc, mybir

    f32 = mybir.dt.float32
    bf16 = mybir.dt.bfloat16
    fp8 = mybir.dt.float8e4
    AF = mybir.ActivationFunctionType
    OP = mybir.AluOpType
    PM = mybir.MatmulPerfMode

    nc = bacc.Bacc()

    SC = S + IPC  # 225: concat of all-j columns and the core's i-slice
    dxT = nc.declare_dram_parameter("xT", [D, B * SC], bf16, isOutput=False)
    dahat = nc.declare_dram_parameter("ahat", [128, 4 * SC], bf16, isOutput=False)
    dbasis = nc.declare_dram_parameter("basis", [128, 1800], bf16, isOutput=False)
    droot = nc.declare_dram_parameter("root", [128, 900], bf16, isOutput=False)
    dbias = nc.declare_dram_parameter("bias", [128, 3], f32, isOutput=False)
    dw1a = nc.declare_dram_parameter("w1a", [128, 1536], bf16, isOutput=False)
    dw1c = nc.declare_dram_parameter("w1c", [128, 1536], bf16, isOutput=False)
    dw2f8 = nc.declare_dram_parameter("w2f8", [128, 2048], fp8, isOutput=False)
    dw2b = nc.declare_dram_parameter("w2b", [128, 2 * M], bf16, isOutput=False)
    dwp = nc.declare_dram_parameter("wp", [128, 4], bf16, isOutput=False)
    dper = nc.declare_dram_parameter("per", [128, 4 * FPC], bf16, isOutput=False)
    dout = nc.declare_dram_parameter("out", [B * 100, NCOL], f32, isOutput=True)

    DCW = [128, 128, 44]  # D=300 chunks
    JCW = [128, 72]  # S=200 chunks

    with tile.TileContext(nc) as tc:
        with (
            tc.tile_pool(name="persist", bufs=1) as pp,
            tc.tile_pool(name="tpool", bufs=2) as tp,
            tc.tile_pool(name="work", bufs=3) as pwork,
            tc.tile_pool(name="sigp", bufs=2) as psig,
        ):
            def load(name, shape, dt, src):
                t = pp.tile(shape, dt, tag=name, name=name)
                if len(shape) == 3:
                    nc.sync.dma_start(t[:, :, :], src)
                else:
                    nc.sync.dma_start(t[:, :], src)
                return t

            xT = [load(f"xT{c}", [DCW[c], B * SC], bf16,
                       dxT[c * 128 : c * 128 + DCW[c], :]) for c in range(3)]
            basis = load("basis", [128, 1800], bf16, dbasis[:, :])
            root = load("root", [128, 900], bf16, droot[:, :])
            bias = load("bias", [128, 3], f32, dbias[:, :])
            w1a = load("w1a", [128, 1536], bf16, dw1a[:, :])
            w1c = load("w1c", [128, 1536], bf16, dw1c[:, :])
            ahat = load("ahat", [128, 4 * SC], bf16, dahat[:, :])
            # fp8 GEMM2 weights: [s(hi/lo)][n] tiles of [128, 2, 128]
            w28 = [[load(f"w28_{s}{n}", [128, 2, 128], fp8,
                         dw2f8[:, (s * 4 + n) * 256 : (s * 4 + n) * 256 + 256])
                    for n in range(4)] for s in range(2)]
            w2b = load("w2b", [128, 2 * M], bf16, dw2b[:, :])
            wp = load("wp", [128, 4], bf16, dwp[:, :])
            peR = [load(f"peR{mc}", [128, FPC], bf16,
                        dper[:, mc * FPC : (mc + 1) * FPC]) for mc in range(4)]

            hT = [[pp.tile([DCW[ec], SC], bf16, tag=f"hT{b}{ec}", name=f"hT{b}{ec}")
                   for ec in range(3)] for b in range(B)]
            uT = [[pp.tile([128, S], bf16, tag=f"uT{b}{mc}", name=f"uT{b}{mc}")
                   for mc in range(4)] for b in range(B)]
            vT = [[pp.tile([128, IPC], f32, tag=f"vT{b}{mc}", name=f"vT{b}{mc}")
                   for mc in range(4)] for b in range(B)]

            # ---------------- stage A: RGCN h, then u/v ----------
            with tc.tile_pool(name="psA", bufs=2, space="PSUM") as psA:
                t1 = [[[pp.tile([JCW[jc], D], bf16, tag=f"t1_{b}{k}{jc}",
                                name=f"t1_{b}{k}{jc}")
                        for jc in range(2)] for k in range(2)] for b in range(B)]
                # phase 1: all t1 groups (b-independent -> PE never stalls)
                for b in range(B):
                    for k in range(2):
                        for jc in range(2):
                            t1ps = psA.tile([JCW[jc], D], f32, tag="mps", name="t1ps")
                            for dc in range(3):
                                nc.tensor.matmul(
                                    t1ps[:, :],
                                    xT[dc][:, b * SC + jc * 128 : b * SC + jc * 128 + JCW[jc]],
                                    basis[0 : DCW[dc], (k * 3 + dc) * D : (k * 3 + dc) * D + D],
                                    start=(dc == 0), stop=(dc == 2),
                                )
                            nc.vector.tensor_copy(t1[b][k][jc][:, :], t1ps[:, :])
                # phase 2: all h groups
                for b in range(B):
                    for ec in range(3):
                        hps = psA.tile([DCW[ec], SC], f32, tag="hps", name="hps")
                        first = True
                        for k in range(2):
                            for jc in range(2):
                                nc.tensor.matmul(
                                    hps[:, :],
                                    t1[b][k][jc][:, ec * 128 : ec * 128 + DCW[ec]],
                                    ahat[0 : JCW[jc], (k * 2 + jc) * SC : (k * 2 + jc + 1) * SC],
                                    start=first, stop=False)
                                first = False
                        for dc in range(3):
                            nc.tensor.matmul(
                                hps[:, :],
                                root[0 : DCW[dc], dc * D + ec * 128 : dc * D + ec * 128 + DCW[ec]],
                                xT[dc][:, b * SC : (b + 1) * SC],
                                start=False, stop=(dc == 2))
                        nc.scalar.activation(hT[b][ec][:, :], hps[:, :], AF.Identity,
                                             bias=bias[0 : DCW[ec], ec : ec + 1])
                # phase 3: all u/v groups
                for b in range(B):
                    for mc in range(4):
                        ups = psA.tile([128, SC], f32, tag="uvps", name="ups")
                        for ec in range(3):
                            nc.tensor.matmul(
                                ups[:, 0:S],
                                w1a[0 : DCW[ec], ec * M + mc * 128 : ec * M + mc * 128 + 128],
                                hT[b][ec][:, 0:S], start=(ec == 0), stop=False)
                        for ec in range(3):
                            nc.tensor.matmul(
                                ups[:, S:SC],
                                w1c[0 : DCW[ec], ec * M + mc * 128 : ec * M + mc * 128 + 128],
                                hT[b][ec][:, S:SC], start=(ec == 0), stop=(ec == 2))
                        nc.vector.tensor_copy(uT[b][mc][:, :], ups[:, 0:S])
                        nc.vector.tensor_copy(vT[b][mc][:, :], ups[:, S:SC])

            # ---------------- stage B: the pairwise MLP ------------------
            tbs = {}

            def emit_bulk_tt(b):
                # t[b][mc] = u[b][mc] (broadcast over the 25 i-rows) + peR[mc]
                tb = tp.tile([128, 4, FPC], bf16, tag="tb", name=f"tb{b}")
                tbs[b] = tb
                for mc in range(4):
                    nc.vector.tensor_add(
                        tb[:, mc, :].rearrange("p (r j) -> p r j", r=IPC),
                        uT[b][mc][:, :].unsqueeze(1).broadcast_to([128, IPC, S]),
                        peR[mc][:, :].rearrange("p (r j) -> p r j", r=IPC))

            with (
                tc.tile_pool(name="ps2", bufs=5, space="PSUM") as ps2,
                tc.tile_pool(name="pp3", bufs=2, space="PSUM") as pp3,
            ):
                def emit_g3(prev):
                    # GEMM3 for the previous unit: h2 chunks stationary,
                    # wp moving -> out [pairs<=100, 1] into the b's pout col.
                    p_u, p_nch, p_pout, p_rh2 = prev
                    for pc in range(p_nch):
                        col = p_u * 4 + pc
                        for mc in range(4):
                            nc.tensor.matmul(
                                p_pout[0:100, col : col + 1],
                                p_rh2[mc][:, pc * 100 : pc * 100 + 100],
                                wp[:, mc : mc + 1],
                                start=(mc == 0), stop=(mc == 3))

                prev = None
                emit_bulk_tt(0)
                for b in range(B):
                    tb = tbs[b]
                    pout = pp3.tile([128, NCOL], f32, tag="pout", name="pout")
                    for u in range(NU):
                        nil = 2 if u < NU - 1 else 1
                        ncols = nil * S
                        # rh1: fp8 for K-chunks 0,1 (Pool), bf16 for 2,3 (DVE)
                        r8 = pwork.tile([128, 2, 400], fp8, tag="r8", name="r8")
                        rb = [pwork.tile([128, 400], bf16, tag=f"rb{j}", name=f"rb{j}")
                              for j in range(2)]
                        for mc in range(4):
                            for h in range(nil):
                                src = tb[:, mc, u * 400 + h * S : u * 400 + h * S + S]
                                if mc < 2:
                                    nc.gpsimd.tensor_scalar(
                                        out=r8[:, mc, h * S : h * S + S],
                                        in0=src,
                                        scalar1=vT[b][mc][:, 2 * u + h : 2 * u + h + 1],
                                        scalar2=0.0, op0=OP.add, op1=OP.max)
                                else:
                                    nc.vector.tensor_scalar(
                                        out=rb[mc - 2][:, h * S : h * S + S],
                                        in0=src,
                                        scalar1=vT[b][mc][:, 2 * u + h : 2 * u + h + 1],
                                        scalar2=0.0, op0=OP.add, op1=OP.max)
                        # GEMM2 + relu2
                        rh2 = [pwork.tile([128, 400], bf16, tag=f"rh2_{n}", name=f"rh2_{n}")
                               for n in range(4)]
                        for n in range(4):
                            ops = ps2.tile([128, 400], f32, tag="ops", name="ops")
                            for s in range(2):
                                nc.tensor.matmul(
                                    ops[:, :ncols],
                                    w28[s][n][:, :, :],
                                    r8[:, :, :ncols],
                                    start=(s == 0), stop=False,
                                    perf_mode=PM.DoubleRow)
                            for j in range(2):
                                nc.tensor.matmul(
                                    ops[:, :ncols],
                                    w2b[:, j * M + n * 128 : j * M + n * 128 + 128],
                                    rb[j][:, :ncols],
                                    start=False, stop=(j == 1))
                            if n == 3:
                                nc.vector.tensor_scalar(
                                    out=rh2[n][:, :ncols], in0=ops[:, :ncols],
                                    scalar1=0.0, scalar2=None, op0=OP.max)
                            else:
                                nc.scalar.activation(rh2[n][:, :ncols], ops[:, :ncols], AF.Relu)
                        # GEMM3 of the PREVIOUS unit (hides relu2 latency)
                        if prev is not None:
                            emit_g3(prev)
                        prev = (u, 4 if nil == 2 else 2, pout, rh2)
                        if u == 6 and b + 1 < B:
                            emit_bulk_tt(b + 1)
                    emit_g3(prev)
                    prev = None
                    sig = psig.tile([128, NCOL], f32, tag="sigb", name="sigb")
                    nc.scalar.activation(sig[0:100, :], pout[0:100, :], AF.Sigmoid)
                    nc.sync.dma_start(dout[b * 100 : b * 100 + 100, :], sig[0:100, :])

    nc.compile()
    return nc


def _host_prep(x, pe_k, pe_v, comp, basis, root, rgcn_bias, W1):
    import ml_dtypes

    bf = ml_dtypes.bfloat16
    f8 = ml_dtypes.float8_e4m3

    ra = _rel_adj(S) % NREL
    onehot = (ra[None, :, :] == np.arange(NREL)[:, None, None]).astype(np.float64)
    deg = onehot.sum(1)
    inv = np.where(deg > 0, 1.0 / np.maximum(deg, 1.0), 0.0)
    anorm = onehot * inv[:, None, :]
    ahat_full = np.einsum("rk,rij->kij", np.asarray(comp, np.float64), anorm)
    ahat_full = ahat_full.astype(np.float32)  # [2, S, S]
    pos = np.clip(np.arange(S)[:, None] - np.arange(S)[None, :] + 1, 0, MAXL)

    x = np.asarray(x, np.float32)
    W1 = np.asarray(W1, np.float32)
    W1a, W1b = W1[:D], W1[D : D + P]
    W1c, W1d = W1[D + P : 2 * D + P], W1[2 * D + P :]
    ttab = (np.asarray(pe_k, np.float64) @ W1b.astype(np.float64)
            + np.asarray(pe_v, np.float64) @ W1d.astype(np.float64)).astype(np.float32)
    ttab_b = ttab.astype(bf).astype(np.float32)  # [11, 512] as the device sees it

    W2 = np.asarray(W2_GLOBAL, np.float32)
    # fp8 half (K rows 0..255): hi + lo residual pair
    W2hi = W2[:256].astype(f8)
    W2lo = (W2[:256] - W2hi.astype(np.float32)).astype(f8)
    w2f8 = np.zeros((128, 2048), f8)
    for s, Wq in enumerate((W2hi, W2lo)):
        for n in range(4):
            for i in range(2):
                w2f8[:, (s * 4 + n) * 256 + i * 128 : (s * 4 + n) * 256 + i * 128 + 128] = \
                    Wq[i * 128 : i * 128 + 128, n * 128 : n * 128 + 128]
    # bf16 half (K rows 256..511)
    w2b = np.zeros((128, 2 * M), np.float32)
    for j in range(2):
        w2b[:, j * M : (j + 1) * M] = W2[(2 + j) * 128 : (3 + j) * 128, :]

    com = {
        "basis": np.concatenate(
            [_pack_k(np.asarray(basis[k], np.float32)) for k in range(2)], axis=1
        ).astype(bf),
        "root": _pack_k(np.asarray(root, np.float32)).astype(bf),
        "w1a": _pack_k(W1a).astype(bf),
        "w1c": _pack_k(W1c).astype(bf),
        "w2f8": w2f8,
        "w2b": w2b.astype(bf),
        "wp": np.ascontiguousarray(np.asarray(WP_GLOBAL, np.float32)[:, 0]
                                   .reshape(4, 128).T).astype(bf),
    }
    bias_p = np.zeros((128, 3), np.float32)
    rb = np.asarray(rgcn_bias, np.float32)
    for c in range(3):
        r = min(128, D - c * 128)
        bias_p[:r, c] = rb[c * 128 : c * 128 + r]
    com["bias"] = bias_p

    SC = S + IPC
    xt_all = x.transpose(2, 0, 1)  # [D, B, S]
    per_core = []
    for c in range(NCORES):
        i0 = c * IPC
        m = dict(com)
        xtc = np.empty((D, B * SC), np.float32)
        for b in range(B):
            xtc[:, b * SC : b * SC + S] = xt_all[:, b, :]
            xtc[:, b * SC + S : (b + 1) * SC] = xt_all[:, b, i0 : i0 + IPC]
        m["xT"] = xtc.astype(bf)
        ah = np.zeros((128, 4 * SC), np.float32)
        for k in range(2):
            for jc in range(2):
                r = 128 if jc == 0 else 72
                base = (k * 2 + jc) * SC
                ah[:r, base : base + S] = ahat_full[k, jc * 128 : jc * 128 + r, :]
                ah[:r, base + S : base + SC] = ahat_full[k, jc * 128 : jc * 128 + r, i0 : i0 + IPC]
        m["ahat"] = ah.astype(bf)
        # peR: [128, mc*FPC + pair] = T[pos(pair), mc*128+p]
        pe_pairs = ttab_b[pos[i0 : i0 + IPC, :].reshape(-1)]  # [FPC, 512]
        m["per"] = np.ascontiguousarray(
            pe_pairs.T.reshape(4, 128, FPC).reshape(512, FPC)
            .reshape(4, 128, FPC).transpose(1, 0, 2).reshape(128, 4 * FPC)).astype(bf)
        per_core.append(m)
    return per_core


W2_GLOBAL = None
WP_GLOBAL = None


def kernel(x, mask, pe_k, pe_v, comp, basis, root, rgcn_bias, W1, W2, Wp,
           _want_results=False, _trace=False):
    global W2_GLOBAL, WP_GLOBAL
    W2_GLOBAL, WP_GLOBAL = W2, Wp

    from concourse.bass_utils import run_bass_kernel_spmd

    if "nc" not in _prog_cache:
        _prog_cache["nc"] = _build_program()
    nc = _prog_cache["nc"]

    in_maps = _host_prep(x, pe_k, pe_v, comp, basis, root, rgcn_bias, W1)
    res = run_bass_kernel_spmd(nc, in_maps, core_ids=list(range(NCORES)),
                               trace=_trace)

    out = np.zeros((B, S, S), np.float32)
    for c in range(NCORES):
        i0 = c * IPC
        arr = np.asarray(res.results[c]["out"], np.float32).reshape(B, 100, NCOL)
        out[:, i0 : i0 + IPC, :] = arr.transpose(0, 2, 1).reshape(B, IPC, S)
    out *= np.asarray(mask, np.float32)
    if _want_results:
        return out, res
    return out


# revision 7
# speedup vs baseline: 1.9817x; 1.0420x over previous
"""Trainium2 Bass kernel for nn_CausePredictor (RGCN + pairwise MLP).

Sharding: data-parallel over the pairwise row index i (dim 1 of the
[B,S,S] output): 8 cores x 25 rows, replicated over B=4.  All per-core
differences are encoded as input DATA (column slices / gathered pe
tables), so one SPMD program serves all cores.

Math (matching reference.py):
  h   = sum_k Ahat_k.T @ (x[b] @ basis_k) + x[b] @ root + bias
  u   = h @ W1a   (j term),  v = h @ W1c  (i term)
  T   = pe_k @ W1b + pe_v @ W1d          # [11, 512], host precomputed
  h1[b,i,j,:] = u[b,j] + v[b,i] + T[pos(i,j)]
  out = sigmoid(Wp . relu(relu(h1) @ W2)) * mask

Performance structure (per TimelineSim cost model):
  - stage A (RGCN + u/v) runs in bf16 (fp32 matmuls are 4x slower).
  - peR = T[pos] pair-expansion is precomputed on HOST and DMA'd in.
  - t = u + peR is ONE broadcast tensor_add per (b, mc) on DVE.
  - rh1 = relu(t + v_i): K-chunks 0,1 quantize to fp8 (on Pool),
    chunks 2,3 stay bf16 (on DVE).
  - GEMM2: fp8 chunks use DoubleRow matmuls (0.5 cyc/row) with W2
    split into hi+lo fp8 pair (weight quantization ~exact); bf16
    chunks use normal matmuls.  3 cyc/row total vs 4 for pure bf16.
  - relu2 PSUM->SBUF copies split 3:1 between Activation and Pool.
  - GEMM3 (the Wp dot) runs with h2 chunks STATIONARY and wp moving:
    out is [pairs<=100, 1], ~1 PE row per matmul instead of 400.
"""

import sys

sys.path.insert(0, "/opt/trn_rl_repo")

import numpy as np

B, S, D, M, P = 4, 200, 300, 512, 100
NREL, MAXL = 9, 10
NCORES = 8
IPC = S // NCORES  # 25 rows of i per core
NU = IPC // 2 + 1  # 13 units per b: 12x 2-row + 1x 1-row
FPC = IPC * S  # 5000 pairs per (b, core)
NCOL = 50  # output columns per b: each col = 100 pairs

_prog_cache = {}


def _rel_adj(s):
    ra = np.arange(s)[None, :] - np.arange(s)[:, None]
    for i in range(s):
        ra[i, i + 1 :] = 1
        num = 1
        for o in range(i - 1, -1, -2):
            ra[i, o] = -num
            if o - 1 >= 0:
                ra[i, o - 1] = -num
            num += 1
        ra[i, :i] = np.maximum(ra[i, :i], -8)  # -(WINDOW+1), WINDOW=7
    return ra


def _pack_k(w, width=None):
    """[K, N] -> [128, ceil(K/128)*N], K chunked onto partitions, zero pad."""
    k, n = w.shape
    nch = (k + 127) // 128
    out = np.zeros((128, nch * n), np.float32)
    for c in range(nch):
        r = min(128, k - c * 128)
        out[:r, c * n : c * n + n] = w[c * 128 : c * 128 + r]
    return out


def _build_program():
    import ml_dtypes  # noqa: F401
    import concourse.tile as tile
    from concourse import bacc, mybir

    f32 = mybir.dt.float32
    bf16 = mybir.dt.bfloat16
    fp8 = mybir.dt.float8e4
    AF = mybir.ActivationFunctionType
    OP = mybir.AluOpType
    PM = mybir.MatmulPerfMode

    nc = bacc.Bacc()

    SC = S + IPC  # 225: concat of all-j columns and the core's i-slice
    dxT = nc.declare_dram_parameter("xT", [D, B * SC], bf16, isOutput=False)
    dahat = nc.declare_dram_parameter("ahat", [128, 4 * SC], bf16, isOutput=False)
    dbasis = nc.declare_dram_parameter("basis", [128, 1800], bf16, isOutput=False)
    droot = nc.declare_dram_parameter("root", [128, 900], bf16, isOutput=False)
    dbias = nc.declare_dram_parameter("bias", [128, 3], f32, isOutput=False)
    dw1a = nc.declare_dram_parameter("w1a", [128, 1536], bf16, isOutput=False)
    dw1c = nc.declare_dram_parameter("w1c", [128, 1536], bf16, isOutput=False)
    dw2f8 = nc.declare_dram_parameter("w2f8", [128, 2048], fp8, isOutput=False)
    dw2b = nc.declare_dram_parameter("w2b", [128, 2 * M], bf16, isOutput=False)
    dwp = nc.declare_dram_parameter("wp", [128, 4], bf16, isOutput=False)
    dper = nc.declare_dram_parameter("per", [128, 4 * FPC], bf16, isOutput=False)
    dout = nc.declare_dram_parameter("out", [B * 100, NCOL], f32, isOutput=True)

    DCW = [128, 128, 44]  # D=300 chunks
    JCW = [128, 72]  # S=200 chunks

    with tile.TileContext(nc) as tc:
        with (
            tc.tile_pool(name="persist", bufs=1) as pp,
            tc.tile_pool(name="tpool", bufs=2) as tp,
            tc.tile_pool(name="work", bufs=3) as pwork,
            tc.tile_pool(name="sigp", bufs=2) as psig,
        ):
            def load(name, shape, dt, src):
                t = pp.tile(shape, dt, tag=name, name=name)
                if len(shape) == 3:
                    nc.sync.dma_start(t[:, :, :], src)
                else:
                    nc.sync.dma_start(t[:, :], src)
                return t

            xT = [load(f"xT{c}", [DCW[c], B * SC], bf16,
                       dxT[c * 128 : c * 128 + DCW[c], :]) for c in range(3)]
            basis = load("basis", [128, 1800], bf16, dbasis[:, :])
            root = load("root", [128, 900], bf16, droot[:, :])
            bias = load("bias", [128, 3], f32, dbias[:, :])
            w1a = load("w1a", [128, 1536], bf16, dw1a[:, :])
            w1c = load("w1c", [128, 1536], bf16, dw1c[:, :])
            ahat = load("ahat", [128, 4 * SC], bf16, dahat[:, :])
            # fp8 GEMM2 weights: [s(hi/lo)][n] tiles of [128, 2, 128]
            w28 = [[load(f"w28_{s}{n}", [128, 2, 128], fp8,
                         dw2f8[:, (s * 4 + n) * 256 : (s * 4 + n) * 256 + 256])
                    for n in range(4)] for s in range(2)]
            w2b = load("w2b", [128, 2 * M], bf16, dw2b[:, :])
            wp = load("wp", [128, 4], bf16, dwp[:, :])
            peR = [load(f"peR{mc}", [128, FPC], bf16,
                        dper[:, mc * FPC : (mc + 1) * FPC]) for mc in range(4)]

            hT = [[pp.tile([DCW[ec], SC], bf16, tag=f"hT{b}{ec}", name=f"hT{b}{ec}")
                   for ec in range(3)] for b in range(B)]
            uT = [[pp.tile([128, S], bf16, tag=f"uT{b}{mc}", name=f"uT{b}{mc}")
                   for mc in range(4)] for b in range(B)]
            vT = [[pp.tile([128, IPC], f32, tag=f"vT{b}{mc}", name=f"vT{b}{mc}")
                   for mc in range(4)] for b in range(B)]

            # ---------------- stage A: RGCN h, then u/v ----------
            with tc.tile_pool(name="psA", bufs=2, space="PSUM") as psA:
                t1 = [[[pp.tile([JCW[jc], D], bf16, tag=f"t1_{b}{k}{jc}",
                                name=f"t1_{b}{k}{jc}")
                        for jc in range(2)] for k in range(2)] for b in range(B)]
                # phase 1: all t1 groups (b-independent -> PE never stalls)
                for b in range(B):
                    for k in range(2):
                        for jc in range(2):
                            t1ps = psA.tile([JCW[jc], D], f32, tag="mps", name="t1ps")
                            for dc in range(3):
                                nc.tensor.matmul(
                                    t1ps[:, :],
                                    xT[dc][:, b * SC + jc * 128 : b * SC + jc * 128 + JCW[jc]],
                                    basis[0 : DCW[dc], (k * 3 + dc) * D : (k * 3 + dc) * D + D],
                                    start=(dc == 0), stop=(dc == 2),
                                )
                            nc.scalar.activation(t1[b][k][jc][:, :], t1ps[:, :], AF.Copy)
                # phase 2: all h groups
                for b in range(B):
                    for ec in range(3):
                        hps = psA.tile([DCW[ec], SC], f32, tag="hps", name="hps")
                        first = True
                        for k in range(2):
                            for jc in range(2):
                                nc.tensor.matmul(
                                    hps[:, :],
                                    t1[b][k][jc][:, ec * 128 : ec * 128 + DCW[ec]],
                                    ahat[0 : JCW[jc], (k * 2 + jc) * SC : (k * 2 + jc + 1) * SC],
                                    start=first, stop=False)
                                first = False
                        for dc in range(3):
                            nc.tensor.matmul(
                                hps[:, :],
                                root[0 : DCW[dc], dc * D + ec * 128 : dc * D + ec * 128 + DCW[ec]],
                                xT[dc][:, b * SC : (b + 1) * SC],
                                start=False, stop=(dc == 2))
                        nc.scalar.activation(hT[b][ec][:, :], hps[:, :], AF.Identity,
                                             bias=bias[0 : DCW[ec], ec : ec + 1])
                # phase 3: all u/v groups
                for b in range(B):
                    for mc in range(4):
                        ups = psA.tile([128, SC], f32, tag="uvps", name="ups")
                        for ec in range(3):
                            nc.tensor.matmul(
                                ups[:, 0:S],
                                w1a[0 : DCW[ec], ec * M + mc * 128 : ec * M + mc * 128 + 128],
                                hT[b][ec][:, 0:S], start=(ec == 0), stop=False)
                        for ec in range(3):
                            nc.tensor.matmul(
                                ups[:, S:SC],
                                w1c[0 : DCW[ec], ec * M + mc * 128 : ec * M + mc * 128 + 128],
                                hT[b][ec][:, S:SC], start=(ec == 0), stop=(ec == 2))
                        nc.scalar.activation(uT[b][mc][:, :], ups[:, 0:S], AF.Copy)
                        nc.scalar.activation(vT[b][mc][:, :], ups[:, S:SC], AF.Copy)

            # ---------------- stage B: the pairwise MLP ------------------
            tbs = {}

            def emit_bulk_tt(b):
                # t[b][mc] = u[b][mc] (broadcast over the 25 i-rows) + peR[mc]
                tb = tp.tile([128, 4, FPC], bf16, tag="tb", name=f"tb{b}")
                tbs[b] = tb
                for mc in range(4):
                    nc.vector.tensor_add(
                        tb[:, mc, :].rearrange("p (r j) -> p r j", r=IPC),
                        uT[b][mc][:, :].unsqueeze(1).broadcast_to([128, IPC, S]),
                        peR[mc][:, :].rearrange("p (r j) -> p r j", r=IPC))

            with (
                tc.tile_pool(name="ps2", bufs=5, space="PSUM") as ps2,
                tc.tile_pool(name="pp3", bufs=2, space="PSUM") as pp3,
            ):
                def emit_g3(prev):
                    # GEMM3 for the previous unit: h2 chunks stationary,
                    # wp moving -> out [pairs<=100, 1] into the b's pout col.
                    p_u, p_nch, p_pout, p_rh2 = prev
                    for pc in range(p_nch):
                        col = p_u * 4 + pc
                        for mc in range(4):
                            nc.tensor.matmul(
                                p_pout[0:100, col : col + 1],
                                p_rh2[mc][:, pc * 100 : pc * 100 + 100],
                                wp[:, mc : mc + 1],
                                start=(mc == 0), stop=(mc == 3))

                prev = None
                emit_bulk_tt(0)
                for b in range(B):
                    tb = tbs[b]
                    pout = pp3.tile([128, NCOL], f32, tag="pout", name="pout")
                    for u in range(NU):
                        nil = 2 if u < NU - 1 else 1
                        ncols = nil * S
                        # rh1: fp8 for K-chunks 0,1 (Pool), bf16 for 2,3 (DVE)
                        r8 = pwork.tile([128, 2, 400], fp8, tag="r8", name="r8")
                        rb = [pwork.tile([128, 400], bf16, tag=f"rb{j}", name=f"rb{j}")
                              for j in range(2)]
                        for mc in range(4):
                            for h in range(nil):
                                src = tb[:, mc, u * 400 + h * S : u * 400 + h * S + S]
                                if mc < 2:
                                    nc.gpsimd.tensor_scalar(
                                        out=r8[:, mc, h * S : h * S + S],
                                        in0=src,
                                        scalar1=vT[b][mc][:, 2 * u + h : 2 * u + h + 1],
                                        scalar2=0.0, op0=OP.add, op1=OP.max)
                                else:
                                    nc.vector.tensor_scalar(
                                        out=rb[mc - 2][:, h * S : h * S + S],
                                        in0=src,
                                        scalar1=vT[b][mc][:, 2 * u + h : 2 * u + h + 1],
                                        scalar2=0.0, op0=OP.add, op1=OP.max)
                        # GEMM2 + relu2
                        rh2 = [pwork.tile([128, 400], bf16, tag=f"rh2_{n}", name=f"rh2_{n}")
                               for n in range(4)]
                        for n in range(4):
                            ops = ps2.tile([128, 400], f32, tag="ops", name="ops")
                            for s in range(2):
                                nc.tensor.matmul(
                                    ops[:, :ncols],
                                    w28[s][n][:, :, :],
                                    r8[:, :, :ncols],
                                    start=(s == 0), stop=False,
                                    perf_mode=PM.DoubleRow)
                            for j in range(2):
                                nc.tensor.matmul(
                                    ops[:, :ncols],
                                    w2b[:, j * M + n * 128 : j * M + n * 128 + 128],
                                    rb[j][:, :ncols],
                                    start=False, stop=(j == 1))
                            if n == 3:
                                nc.vector.tensor_scalar(
                                    out=rh2[n][:, :ncols], in0=ops[:, :ncols],
                                    scalar1=0.0, scalar2=None, op0=OP.max)
                            else:
                                nc.scalar.activation(rh2[n][:, :ncols], ops[:, :ncols], AF.Relu)
                        # GEMM3 of the PREVIOUS unit (hides relu2 latency)
                        if prev is not None:
                            emit_g3(prev)
                        prev = (u, 4 if nil == 2 else 2, pout, rh2)
                        if u == 6 and b + 1 < B:
                            emit_bulk_tt(b + 1)
                    emit_g3(prev)
                    prev = None
                    sig = psig.tile([128, NCOL], f32, tag="sigb", name="sigb")
                    nc.scalar.activation(sig[0:100, :], pout[0:100, :], AF.Sigmoid)
                    nc.sync.dma_start(dout[b * 100 : b * 100 + 100, :], sig[0:100, :])

    nc.compile()
    return nc


def _host_prep(x, pe_k, pe_v, comp, basis, root, rgcn_bias, W1):
    import ml_dtypes

    bf = ml_dtypes.bfloat16
    f8 = ml_dtypes.float8_e4m3

    ra = _rel_adj(S) % NREL
    onehot = (ra[None, :, :] == np.arange(NREL)[:, None, None]).astype(np.float64)
    deg = onehot.sum(1)
    inv = np.where(deg > 0, 1.0 / np.maximum(deg, 1.0), 0.0)
    anorm = onehot * inv[:, None, :]
    ahat_full = np.einsum("rk,rij->kij", np.asarray(comp, np.float64), anorm)
    ahat_full = ahat_full.astype(np.float32)  # [2, S, S]
    pos = np.clip(np.arange(S)[:, None] - np.arange(S)[None, :] + 1, 0, MAXL)

    x = np.asarray(x, np.float32)
    W1 = np.asarray(W1, np.float32)
    W1a, W1b = W1[:D], W1[D : D + P]
    W1c, W1d = W1[D + P : 2 * D + P], W1[2 * D + P :]
    ttab = (np.asarray(pe_k, np.float64) @ W1b.astype(np.float64)
            + np.asarray(pe_v, np.float64) @ W1d.astype(np.float64)).astype(np.float32)
    ttab_b = ttab.astype(bf).astype(np.float32)  # [11, 512] as the device sees it

    W2 = np.asarray(W2_GLOBAL, np.float32)
    # fp8 half (K rows 0..255): hi + lo residual pair
    W2hi = W2[:256].astype(f8)
    W2lo = (W2[:256] - W2hi.astype(np.float32)).astype(f8)
    w2f8 = np.zeros((128, 2048), f8)
    for s, Wq in enumerate((W2hi, W2lo)):
        for n in range(4):
            for i in range(2):
                w2f8[:, (s * 4 + n) * 256 + i * 128 : (s * 4 + n) * 256 + i * 128 + 128] = \
                    Wq[i * 128 : i * 128 + 128, n * 128 : n * 128 + 128]
    # bf16 half (K rows 256..511)
    w2b = np.zeros((128, 2 * M), np.float32)
    for j in range(2):
        w2b[:, j * M : (j + 1) * M] = W2[(2 + j) * 128 : (3 + j) * 128, :]

    com = {
        "basis": np.concatenate(
            [_pack_k(np.asarray(basis[k], np.float32)) for k in range(2)], axis=1
        ).astype(bf),
        "root": _pack_k(np.asarray(root, np.float32)).astype(bf),
        "w1a": _pack_k(W1a).astype(bf),
        "w1c": _pack_k(W1c).astype(bf),
        "w2f8": w2f8,
        "w2b": w2b.astype(bf),
        "wp": np.ascontiguousarray(np.asarray(WP_GLOBAL, np.float32)[:, 0]
                                   .reshape(4, 128).T).astype(bf),
    }
    bias_p = np.zeros((128, 3), np.float32)
    rb = np.asarray(rgcn_bias, np.float32)
    for c in range(3):
        r = min(128, D - c * 128)
        bias_p[:r, c] = rb[c * 128 : c * 128 + r]
    com["bias"] = bias_p

    SC = S + IPC
    xt_all = x.transpose(2, 0, 1)  # [D, B, S]
    per_core = []
    for c in range(NCORES):
        i0 = c * IPC
        m = dict(com)
        xtc = np.empty((D, B * SC), np.float32)
        for b in range(B):
            xtc[:, b * SC : b * SC + S] = xt_all[:, b, :]
            xtc[:, b * SC + S : (b + 1) * SC] = xt_all[:, b, i0 : i0 + IPC]
        m["xT"] = xtc.astype(bf)
        ah = np.zeros((128, 4 * SC), np.float32)
        for k in range(2):
            for jc in range(2):
                r = 128 if jc == 0 else 72
                base = (k * 2 + jc) * SC
                ah[:r, base : base + S] = ahat_full[k, jc * 128 : jc * 128 + r, :]
                ah[:r, base + S : base + SC] = ahat_full[k, jc * 128 : jc * 128 + r, i0 : i0 + IPC]
        m["ahat"] = ah.astype(bf)
        # peR: [128, mc*FPC + pair] = T[pos(pair), mc*128+p]
        pe_pairs = ttab_b[pos[i0 : i0 + IPC, :].reshape(-1)]  # [FPC, 512]
        m["per"] = np.ascontiguousarray(
            pe_pairs.T.reshape(4, 128, FPC).reshape(512, FPC)
            .reshape(4, 128, FPC).transpose(1, 0, 2).reshape(128, 4 * FPC)).astype(bf)
        per_core.append(m)
    return per_core


W2_GLOBAL = None
WP_GLOBAL = None


def kernel(x, mask, pe_k, pe_v, comp, basis, root, rgcn_bias, W1, W2, Wp,
           _want_results=False, _trace=False):
    global W2_GLOBAL, WP_GLOBAL
    W2_GLOBAL, WP_GLOBAL = W2, Wp

    from concourse.bass_utils import run_bass_kernel_spmd

    if "nc" not in _prog_cache:
        _prog_cache["nc"] = _build_program()
    nc = _prog_cache["nc"]

    in_maps = _host_prep(x, pe_k, pe_v, comp, basis, root, rgcn_bias, W1)
    res = run_bass_kernel_spmd(nc, in_maps, core_ids=list(range(NCORES)),
                               trace=_trace)

    out = np.zeros((B, S, S), np.float32)
    for c in range(NCORES):
        i0 = c * IPC
        arr = np.asarray(res.results[c]["out"], np.float32).reshape(B, 100, NCOL)
        out[:, i0 : i0 + IPC, :] = arr.transpose(0, 2, 1).reshape(B, IPC, S)
    out *= np.asarray(mask, np.float32)
    if _want_results:
        return out, res
    return out


# revision 11
# speedup vs baseline: 2.0474x; 1.0331x over previous
"""Trainium2 Bass kernel for nn_CausePredictor (RGCN + pairwise MLP).

Sharding: data-parallel over the pairwise row index i (dim 1 of the
[B,S,S] output): 8 cores x 25 rows, replicated over B=4.  All per-core
differences are encoded as input DATA (column slices / gathered pe
tables), so one SPMD program serves all cores.

Math (matching reference.py):
  h   = sum_k Ahat_k.T @ (x[b] @ basis_k) + x[b] @ root + bias
  u   = h @ W1a   (j term),  v = h @ W1c  (i term)
  T   = pe_k @ W1b + pe_v @ W1d          # [11, 512], host precomputed
  h1[b,i,j,:] = u[b,j] + v[b,i] + T[pos(i,j)]
  out = sigmoid(Wp . relu(relu(h1) @ W2)) * mask

Performance structure (per TimelineSim cost model):
  - stage A (RGCN + u/v) runs in bf16 (fp32 matmuls are 4x slower).
  - peR = T[pos] pair-expansion is precomputed on HOST and DMA'd in.
  - t = u + peR is ONE broadcast tensor_add per (b, mc) on DVE.
  - rh1 = relu(t + v_i): K-chunks 0,1 quantize to fp8 (on Pool),
    chunks 2,3 stay bf16 (on DVE).
  - GEMM2: fp8 chunks use DoubleRow matmuls (0.5 cyc/row) with W2
    split into hi+lo fp8 pair (weight quantization ~exact); bf16
    chunks use normal matmuls.  3 cyc/row total vs 4 for pure bf16.
  - relu2 PSUM->SBUF copies split 3:1 between Activation and Pool.
  - GEMM3 (the Wp dot) runs with h2 chunks STATIONARY and wp moving:
    out is [pairs<=100, 1], ~1 PE row per matmul instead of 400.
"""

import sys

sys.path.insert(0, "/opt/trn_rl_repo")

import numpy as np

B, S, D, M, P = 4, 200, 300, 512, 100
NREL, MAXL = 9, 10
NCORES = 8
IPC = S // NCORES  # 25 rows of i per core
NU = IPC // 2 + 1  # 13 units per b: 12x 2-row + 1x 1-row
FPC = IPC * S  # 5000 pairs per (b, core)
NCOL = 50  # output columns per b: each col = 100 pairs

_prog_cache = {}


def _rel_adj(s):
    ra = np.arange(s)[None, :] - np.arange(s)[:, None]
    for i in range(s):
        ra[i, i + 1 :] = 1
        num = 1
        for o in range(i - 1, -1, -2):
            ra[i, o] = -num
            if o - 1 >= 0:
                ra[i, o - 1] = -num
            num += 1
        ra[i, :i] = np.maximum(ra[i, :i], -8)  # -(WINDOW+1), WINDOW=7
    return ra


def _pack_k(w, width=None):
    """[K, N] -> [128, ceil(K/128)*N], K chunked onto partitions, zero pad."""
    k, n = w.shape
    nch = (k + 127) // 128
    out = np.zeros((128, nch * n), np.float32)
    for c in range(nch):
        r = min(128, k - c * 128)
        out[:r, c * n : c * n + n] = w[c * 128 : c * 128 + r]
    return out


def _build_program():
    import ml_dtypes  # noqa: F401
    import concourse.tile as tile
    from concourse import bacc, mybir

    f32 = mybir.dt.float32
    bf16 = mybir.dt.bfloat16
    fp8 = mybir.dt.float8e4
    AF = mybir.ActivationFunctionType
    OP = mybir.AluOpType
    PM = mybir.MatmulPerfMode

    nc = bacc.Bacc()

    SC = S + IPC  # 225: concat of all-j columns and the core's i-slice
    dxT = nc.declare_dram_parameter("xT", [D, B * SC], bf16, isOutput=False)
    dahat = nc.declare_dram_parameter("ahat", [128, 4 * SC], bf16, isOutput=False)
    dbasis = nc.declare_dram_parameter("basis", [128, 1800], bf16, isOutput=False)
    droot = nc.declare_dram_parameter("root", [128, 900], bf16, isOutput=False)
    dbias = nc.declare_dram_parameter("bias", [128, 3], f32, isOutput=False)
    dw1a = nc.declare_dram_parameter("w1a", [128, 1536], bf16, isOutput=False)
    dw1c = nc.declare_dram_parameter("w1c", [128, 1536], bf16, isOutput=False)
    dw2f8 = nc.declare_dram_parameter("w2f8", [128, 2048], fp8, isOutput=False)
    dw2b = nc.declare_dram_parameter("w2b", [128, 2 * M], bf16, isOutput=False)
    dwp = nc.declare_dram_parameter("wp", [128, 4], bf16, isOutput=False)
    dper = nc.declare_dram_parameter("per", [128, 4 * FPC], bf16, isOutput=False)
    dout = nc.declare_dram_parameter("out", [B * 100, NCOL], f32, isOutput=True)

    DCW = [128, 128, 44]  # D=300 chunks
    JCW = [128, 72]  # S=200 chunks

    with tile.TileContext(nc) as tc:
        with (
            tc.tile_pool(name="persist", bufs=1) as pp,
            tc.tile_pool(name="tpool", bufs=2) as tp,
            tc.tile_pool(name="work", bufs=3) as pwork,
            tc.tile_pool(name="sigp", bufs=2) as psig,
        ):
            def load(name, shape, dt, src):
                t = pp.tile(shape, dt, tag=name, name=name)
                if len(shape) == 3:
                    nc.sync.dma_start(t[:, :, :], src)
                else:
                    nc.sync.dma_start(t[:, :], src)
                return t

            # DMA order = consumption order: t1 needs basis+xT, h needs
            # ahat/root/bias, u/v needs w1a/w1c, bulk-TT needs peR chunks,
            # GEMM2 needs w2 tiles last.
            basis = load("basis", [128, 1800], bf16, dbasis[:, :])
            xT = [load(f"xT{c}", [DCW[c], B * SC], bf16,
                       dxT[c * 128 : c * 128 + DCW[c], :]) for c in range(3)]
            ahat = load("ahat", [128, 4 * SC], bf16, dahat[:, :])
            root = load("root", [128, 900], bf16, droot[:, :])
            bias = load("bias", [128, 3], f32, dbias[:, :])
            w1a = load("w1a", [128, 1536], bf16, dw1a[:, :])
            w1c = load("w1c", [128, 1536], bf16, dw1c[:, :])
            peR = [load(f"peR{mc}", [128, FPC], bf16,
                        dper[:, mc * FPC : (mc + 1) * FPC]) for mc in range(4)]
            # fp8 GEMM2 weights: one tile, sliced [128, 2, 128] per (s, n)
            w28t = load("w28t", [128, 2048], fp8, dw2f8[:, :])
            w28 = [[w28t[:, (s * 4 + n) * 256 : (s * 4 + n) * 256 + 256]
                    .rearrange("p (two m) -> p two m", two=2)
                    for n in range(4)] for s in range(2)]
            w2b = load("w2b", [128, 2 * M], bf16, dw2b[:, :])
            wp = load("wp", [128, 4], bf16, dwp[:, :])

            hT = [[pp.tile([DCW[ec], SC], bf16, tag=f"hT{b}{ec}", name=f"hT{b}{ec}")
                   for ec in range(3)] for b in range(B)]
            uT = [[pp.tile([128, S], bf16, tag=f"uT{b}{mc}", name=f"uT{b}{mc}")
                   for mc in range(4)] for b in range(B)]
            vT = [[pp.tile([128, IPC], f32, tag=f"vT{b}{mc}", name=f"vT{b}{mc}")
                   for mc in range(4)] for b in range(B)]

            # bulk-TT parts: t[b][mc] = u[b][mc] (broadcast over i) + peR[mc],
            # emitted in 8 half-row pieces so DVE is never blocked for long.
            tbs = {}
            HROWS = [(0, 13), (13, IPC)]

            def emit_tt_part(b, part):
                if b not in tbs:
                    tbs[b] = tp.tile([128, 4, FPC], bf16, tag="tb", name=f"tb{b}")
                tb = tbs[b]
                half, mc = divmod(part, 4)
                r0, r1 = HROWS[half]
                rows = r1 - r0
                nc.vector.tensor_add(
                    tb[:, mc, r0 * S : r1 * S].rearrange("p (r j) -> p r j", r=rows),
                    uT[b][mc][:, :].unsqueeze(1).broadcast_to([128, rows, S]),
                    peR[mc][:, r0 * S : r1 * S].rearrange("p (r j) -> p r j", r=rows))

            # ---------------- stage A: RGCN h, then u/v ----------
            with tc.tile_pool(name="psA", bufs=2, space="PSUM") as psA:
                t1 = [[[pp.tile([JCW[jc], D], bf16, tag=f"t1_{b}{k}{jc}",
                                name=f"t1_{b}{k}{jc}")
                        for jc in range(2)] for k in range(2)] for b in range(B)]

                def emit_t1(b):
                    for k in range(2):
                        for jc in range(2):
                            t1ps = psA.tile([JCW[jc], D], f32, tag="mps", name="t1ps")
                            for dc in range(3):
                                nc.tensor.matmul(
                                    t1ps[:, :],
                                    xT[dc][:, b * SC + jc * 128 : b * SC + jc * 128 + JCW[jc]],
                                    basis[0 : DCW[dc], (k * 3 + dc) * D : (k * 3 + dc) * D + D],
                                    start=(dc == 0), stop=(dc == 2),
                                )
                            nc.scalar.activation(t1[b][k][jc][:, :], t1ps[:, :], AF.Copy)

                def emit_h(b):
                    for ec in range(3):
                        hps = psA.tile([DCW[ec], SC], f32, tag="hps", name="hps")
                        first = True
                        for k in range(2):
                            for jc in range(2):
                                nc.tensor.matmul(
                                    hps[:, :],
                                    t1[b][k][jc][:, ec * 128 : ec * 128 + DCW[ec]],
                                    ahat[0 : JCW[jc], (k * 2 + jc) * SC : (k * 2 + jc + 1) * SC],
                                    start=first, stop=False)
                                first = False
                        for dc in range(3):
                            nc.tensor.matmul(
                                hps[:, :],
                                root[0 : DCW[dc], dc * D + ec * 128 : dc * D + ec * 128 + DCW[ec]],
                                xT[dc][:, b * SC : (b + 1) * SC],
                                start=False, stop=(dc == 2))
                        nc.scalar.activation(hT[b][ec][:, :], hps[:, :], AF.Identity,
                                             bias=bias[0 : DCW[ec], ec : ec + 1])

                def emit_uv(b):
                    for mc in range(4):
                        ups = psA.tile([128, SC], f32, tag="uvps", name="ups")
                        for ec in range(3):
                            nc.tensor.matmul(
                                ups[:, 0:S],
                                w1a[0 : DCW[ec], ec * M + mc * 128 : ec * M + mc * 128 + 128],
                                hT[b][ec][:, 0:S], start=(ec == 0), stop=False)
                        for ec in range(3):
                            nc.tensor.matmul(
                                ups[:, S:SC],
                                w1c[0 : DCW[ec], ec * M + mc * 128 : ec * M + mc * 128 + 128],
                                hT[b][ec][:, S:SC], start=(ec == 0), stop=(ec == 2))
                        nc.scalar.activation(uT[b][mc][:, :], ups[:, 0:S], AF.Copy)
                        nc.scalar.activation(vT[b][mc][:, :], ups[:, S:SC], AF.Copy)

                # b=0 chain first so its bulk-TT (DVE) overlaps the rest of
                # stage A on the PE.
                emit_t1(0)
                emit_h(0)
                emit_uv(0)
                for part in range(8):
                    emit_tt_part(0, part)
                for b in range(1, B):
                    emit_t1(b)
                for b in range(1, B):
                    emit_h(b)
                for b in range(1, B):
                    emit_uv(b)

            # ---------------- stage B: the pairwise MLP ------------------
            with (
                tc.tile_pool(name="ps2", bufs=5, space="PSUM") as ps2,
                tc.tile_pool(name="pp3", bufs=2, space="PSUM") as pp3,
            ):
                def emit_g3(prev):
                    # GEMM3 for the previous unit: h2 chunks stationary,
                    # wp moving -> out [pairs<=100, 1] into the b's pout col.
                    p_u, p_nch, p_pout, p_rh2 = prev
                    for pc in range(p_nch):
                        col = p_u * 4 + pc
                        for mc in range(4):
                            nc.tensor.matmul(
                                p_pout[0:100, col : col + 1],
                                p_rh2[mc][:, pc * 100 : pc * 100 + 100],
                                wp[:, mc : mc + 1],
                                start=(mc == 0), stop=(mc == 3))

                prev = None
                for b in range(B):
                    tb = tbs[b]
                    pout = pp3.tile([128, NCOL], f32, tag="pout", name="pout")
                    sig = psig.tile([128, NCOL], f32, tag="sigb", name="sigb")
                    for u in range(NU):
                        nil = 2 if u < NU - 1 else 1
                        ncols = nil * S
                        # rh1: fp8 for K-chunks 0,1 (Pool), bf16 for 2,3 (DVE)
                        r8 = pwork.tile([128, 2, 400], fp8, tag="r8", name="r8")
                        rb = [pwork.tile([128, 400], bf16, tag=f"rb{j}", name=f"rb{j}")
                              for j in range(2)]
                        for mc in range(4):
                            for h in range(nil):
                                src = tb[:, mc, u * 400 + h * S : u * 400 + h * S + S]
                                if mc < 2:
                                    nc.gpsimd.tensor_scalar(
                                        out=r8[:, mc, h * S : h * S + S],
                                        in0=src,
                                        scalar1=vT[b][mc][:, 2 * u + h : 2 * u + h + 1],
                                        scalar2=0.0, op0=OP.add, op1=OP.max)
                                else:
                                    nc.vector.tensor_scalar(
                                        out=rb[mc - 2][:, h * S : h * S + S],
                                        in0=src,
                                        scalar1=vT[b][mc][:, 2 * u + h : 2 * u + h + 1],
                                        scalar2=0.0, op0=OP.add, op1=OP.max)
                        # GEMM2 + relu2
                        rh2 = [pwork.tile([128, 400], bf16, tag=f"rh2_{n}", name=f"rh2_{n}")
                               for n in range(4)]
                        for n in range(4):
                            ops = ps2.tile([128, 400], f32, tag="ops", name="ops")
                            for s in range(2):
                                nc.tensor.matmul(
                                    ops[:, :ncols],
                                    w28[s][n][:, :, :],
                                    r8[:, :, :ncols],
                                    start=(s == 0), stop=False,
                                    perf_mode=PM.DoubleRow)
                            for j in range(2):
                                nc.tensor.matmul(
                                    ops[:, :ncols],
                                    w2b[:, j * M + n * 128 : j * M + n * 128 + 128],
                                    rb[j][:, :ncols],
                                    start=False, stop=(j == 1))
                            if n == 3 and u % 2 == 0:
                                nc.vector.tensor_scalar(
                                    out=rh2[n][:, :ncols], in0=ops[:, :ncols],
                                    scalar1=0.0, scalar2=None, op0=OP.max)
                            else:
                                nc.scalar.activation(rh2[n][:, :ncols], ops[:, :ncols], AF.Relu)
                        # GEMM3 of the PREVIOUS unit (hides relu2 latency)
                        if prev is not None:
                            emit_g3(prev)
                        prev = (u, 4 if nil == 2 else 2, pout, rh2)
                        # spread the next batch's bulk-TT parts over units 2..9
                        if 2 <= u <= 9 and b + 1 < B:
                            emit_tt_part(b + 1, u - 2)
                        # first 6 units' output cols are final once g3(5) ran
                        if u == 7:
                            nc.scalar.activation(sig[0:100, 0:24], pout[0:100, 0:24],
                                                 AF.Sigmoid)
                            nc.sync.dma_start(dout[b * 100 : b * 100 + 100, 0:24],
                                              sig[0:100, 0:24])
                    emit_g3(prev)
                    prev = None
                    nc.scalar.activation(sig[0:100, 24:NCOL], pout[0:100, 24:NCOL],
                                         AF.Sigmoid)
                    nc.sync.dma_start(dout[b * 100 : b * 100 + 100, 24:NCOL],
                                      sig[0:100, 24:NCOL])

    nc.compile()
    return nc


def _host_prep(x, pe_k, pe_v, comp, basis, root, rgcn_bias, W1):
    import ml_dtypes

    bf = ml_dtypes.bfloat16
    f8 = ml_dtypes.float8_e4m3

    ra = _rel_adj(S) % NREL
    onehot = (ra[None, :, :] == np.arange(NREL)[:, None, None]).astype(np.float64)
    deg = onehot.sum(1)
    inv = np.where(deg > 0, 1.0 / np.maximum(deg, 1.0), 0.0)
    anorm = onehot * inv[:, None, :]
    ahat_full = np.einsum("rk,rij->kij", np.asarray(comp, np.float64), anorm)
    ahat_full = ahat_full.astype(np.float32)  # [2, S, S]
    pos = np.clip(np.arange(S)[:, None] - np.arange(S)[None, :] + 1, 0, MAXL)

    x = np.asarray(x, np.float32)
    W1 = np.asarray(W1, np.float32)
    W1a, W1b = W1[:D], W1[D : D + P]
    W1c, W1d = W1[D + P : 2 * D + P], W1[2 * D + P :]
    ttab = (np.asarray(pe_k, np.float64) @ W1b.astype(np.float64)
            + np.asarray(pe_v, np.float64) @ W1d.astype(np.float64)).astype(np.float32)
    ttab_b = ttab.astype(bf).astype(np.float32)  # [11, 512] as the device sees it

    W2 = np.asarray(W2_GLOBAL, np.float32)
    # fp8 half (K rows 0..255): hi + lo residual pair
    W2hi = W2[:256].astype(f8)
    W2lo = (W2[:256] - W2hi.astype(np.float32)).astype(f8)
    w2f8 = np.zeros((128, 2048), f8)
    for s, Wq in enumerate((W2hi, W2lo)):
        for n in range(4):
            for i in range(2):
                w2f8[:, (s * 4 + n) * 256 + i * 128 : (s * 4 + n) * 256 + i * 128 + 128] = \
                    Wq[i * 128 : i * 128 + 128, n * 128 : n * 128 + 128]
    # bf16 half (K rows 256..511)
    w2b = np.zeros((128, 2 * M), np.float32)
    for j in range(2):
        w2b[:, j * M : (j + 1) * M] = W2[(2 + j) * 128 : (3 + j) * 128, :]

    com = {
        "basis": np.concatenate(
            [_pack_k(np.asarray(basis[k], np.float32)) for k in range(2)], axis=1
        ).astype(bf),
        "root": _pack_k(np.asarray(root, np.float32)).astype(bf),
        "w1a": _pack_k(W1a).astype(bf),
        "w1c": _pack_k(W1c).astype(bf),
        "w2f8": w2f8,
        "w2b": w2b.astype(bf),
        "wp": np.ascontiguousarray(np.asarray(WP_GLOBAL, np.float32)[:, 0]
                                   .reshape(4, 128).T).astype(bf),
    }
    bias_p = np.zeros((128, 3), np.float32)
    rb = np.asarray(rgcn_bias, np.float32)
    for c in range(3):
        r = min(128, D - c * 128)
        bias_p[:r, c] = rb[c * 128 : c * 128 + r]
    com["bias"] = bias_p

    SC = S + IPC
    xt_all = x.transpose(2, 0, 1)  # [D, B, S]
    per_core = []
    for c in range(NCORES):
        i0 = c * IPC
        m = dict(com)
        xtc = np.empty((D, B * SC), np.float32)
        for b in range(B):
            xtc[:, b * SC : b * SC + S] = xt_all[:, b, :]
            xtc[:, b * SC + S : (b + 1) * SC] = xt_all[:, b, i0 : i0 + IPC]
        m["xT"] = xtc.astype(bf)
        ah = np.zeros((128, 4 * SC), np.float32)
        for k in range(2):
            for jc in range(2):
                r = 128 if jc == 0 else 72
                base = (k * 2 + jc) * SC
                ah[:r, base : base + S] = ahat_full[k, jc * 128 : jc * 128 + r, :]
                ah[:r, base + S : base + SC] = ahat_full[k, jc * 128 : jc * 128 + r, i0 : i0 + IPC]
        m["ahat"] = ah.astype(bf)
        # peR: [128, mc*FPC + pair] = T[pos(pair), mc*128+p]
        pe_pairs = ttab_b[pos[i0 : i0 + IPC, :].reshape(-1)]  # [FPC, 512]
        m["per"] = np.ascontiguousarray(
            pe_pairs.T.reshape(4, 128, FPC).reshape(512, FPC)
            .reshape(4, 128, FPC).transpose(1, 0, 2).reshape(128, 4 * FPC)).astype(bf)
        per_core.append(m)
    return per_core


W2_GLOBAL = None
WP_GLOBAL = None


def kernel(x, mask, pe_k, pe_v, comp, basis, root, rgcn_bias, W1, W2, Wp,
           _want_results=False, _trace=False):
    global W2_GLOBAL, WP_GLOBAL
    W2_GLOBAL, WP_GLOBAL = W2, Wp

    from concourse.bass_utils import run_bass_kernel_spmd

    if "nc" not in _prog_cache:
        _prog_cache["nc"] = _build_program()
    nc = _prog_cache["nc"]

    in_maps = _host_prep(x, pe_k, pe_v, comp, basis, root, rgcn_bias, W1)
    res = run_bass_kernel_spmd(nc, in_maps, core_ids=list(range(NCORES)),
                               trace=_trace)

    out = np.zeros((B, S, S), np.float32)
    for c in range(NCORES):
        i0 = c * IPC
        arr = np.asarray(res.results[c]["out"], np.float32).reshape(B, 100, NCOL)
        out[:, i0 : i0 + IPC, :] = arr.transpose(0, 2, 1).reshape(B, IPC, S)
    out *= np.asarray(mask, np.float32)
    if _want_results:
        return out, res
    return out


# revision 14
# speedup vs baseline: 2.0516x; 1.0021x over previous
"""Trainium2 Bass kernel for nn_CausePredictor (RGCN + pairwise MLP).

Sharding: data-parallel over the pairwise row index i (dim 1 of the
[B,S,S] output): 8 cores x 25 rows, replicated over B=4.  All per-core
differences are encoded as input DATA (column slices / gathered pe
tables), so one SPMD program serves all cores.

Math (matching reference.py):
  h   = sum_k Ahat_k.T @ (x[b] @ basis_k) + x[b] @ root + bias
  u   = h @ W1a   (j term),  v = h @ W1c  (i term)
  T   = pe_k @ W1b + pe_v @ W1d          # [11, 512], host precomputed
  h1[b,i,j,:] = u[b,j] + v[b,i] + T[pos(i,j)]
  out = sigmoid(Wp . relu(relu(h1) @ W2)) * mask

Performance structure (per TimelineSim cost model):
  - stage A (RGCN + u/v) runs in bf16 (fp32 matmuls are 4x slower).
  - peR = T[pos] pair-expansion is precomputed on HOST and DMA'd in.
  - t = u + peR is ONE broadcast tensor_add per (b, mc) on DVE.
  - rh1 = relu(t + v_i): K-chunks 0,1 quantize to fp8 (on Pool),
    chunks 2,3 stay bf16 (on DVE).
  - GEMM2: fp8 chunks use DoubleRow matmuls (0.5 cyc/row) with W2
    split into hi+lo fp8 pair (weight quantization ~exact); bf16
    chunks use normal matmuls.  3 cyc/row total vs 4 for pure bf16.
  - relu2 PSUM->SBUF copies split 3:1 between Activation and Pool.
  - GEMM3 (the Wp dot) runs with h2 chunks STATIONARY and wp moving:
    out is [pairs<=100, 1], ~1 PE row per matmul instead of 400.
"""

import sys

sys.path.insert(0, "/opt/trn_rl_repo")

import numpy as np

B, S, D, M, P = 4, 200, 300, 512, 100
NREL, MAXL = 9, 10
NCORES = 8
IPC = S // NCORES  # 25 rows of i per core
NU = IPC // 2 + 1  # 13 units per b: 12x 2-row + 1x 1-row
FPC = IPC * S  # 5000 pairs per (b, core)
NCOL = 50  # output columns per b: each col = 100 pairs

_prog_cache = {}


def _rel_adj(s):
    ra = np.arange(s)[None, :] - np.arange(s)[:, None]
    for i in range(s):
        ra[i, i + 1 :] = 1
        num = 1
        for o in range(i - 1, -1, -2):
            ra[i, o] = -num
            if o - 1 >= 0:
                ra[i, o - 1] = -num
            num += 1
        ra[i, :i] = np.maximum(ra[i, :i], -8)  # -(WINDOW+1), WINDOW=7
    return ra


def _pack_k(w, width=None):
    """[K, N] -> [128, ceil(K/128)*N], K chunked onto partitions, zero pad."""
    k, n = w.shape
    nch = (k + 127) // 128
    out = np.zeros((128, nch * n), np.float32)
    for c in range(nch):
        r = min(128, k - c * 128)
        out[:r, c * n : c * n + n] = w[c * 128 : c * 128 + r]
    return out


def _build_program():
    import ml_dtypes  # noqa: F401
    import concourse.tile as tile
    from concourse import bacc, mybir

    f32 = mybir.dt.float32
    bf16 = mybir.dt.bfloat16
    fp8 = mybir.dt.float8e4
    AF = mybir.ActivationFunctionType
    OP = mybir.AluOpType
    PM = mybir.MatmulPerfMode

    nc = bacc.Bacc()

    SC = S + IPC  # 225: concat of all-j columns and the core's i-slice
    dxT = nc.declare_dram_parameter("xT", [D, B * SC], bf16, isOutput=False)
    dahat = nc.declare_dram_parameter("ahat", [128, 4 * SC], bf16, isOutput=False)
    dbasis = nc.declare_dram_parameter("basis", [128, 1800], bf16, isOutput=False)
    droot = nc.declare_dram_parameter("root", [128, 900], bf16, isOutput=False)
    dbias = nc.declare_dram_parameter("bias", [128, 3], f32, isOutput=False)
    dw1a = nc.declare_dram_parameter("w1a", [128, 1536], bf16, isOutput=False)
    dw1c = nc.declare_dram_parameter("w1c", [128, 1536], bf16, isOutput=False)
    dw2f8 = nc.declare_dram_parameter("w2f8", [128, 2048], fp8, isOutput=False)
    dw2b = nc.declare_dram_parameter("w2b", [128, 2 * M], bf16, isOutput=False)
    dwp = nc.declare_dram_parameter("wp", [128, 4], bf16, isOutput=False)
    dper = nc.declare_dram_parameter("per", [128, 4 * FPC], bf16, isOutput=False)
    dout = nc.declare_dram_parameter("out", [B * 100, NCOL], f32, isOutput=True)

    DCW = [128, 128, 44]  # D=300 chunks
    JCW = [128, 72]  # S=200 chunks

    with tile.TileContext(nc) as tc:
        with (
            tc.tile_pool(name="persist", bufs=1) as pp,
            tc.tile_pool(name="tpool", bufs=2) as tp,
            tc.tile_pool(name="work", bufs=3) as pwork,
            tc.tile_pool(name="sigp", bufs=2) as psig,
        ):
            def load(name, shape, dt, src):
                t = pp.tile(shape, dt, tag=name, name=name)
                if len(shape) == 3:
                    nc.sync.dma_start(t[:, :, :], src)
                else:
                    nc.sync.dma_start(t[:, :], src)
                return t

            # DMA order = deadline order.  The serial DMA stream (~23us for
            # 8MB) is a startup critical path: stage-A weights first, then
            # the first-half rows of peR (consumed by the early units),
            # then stage-B weights, then peR's second halves (not needed
            # until ~unit 7 of b=0).
            CH = 13 * S  # peR column split matching the bulk-TT halves
            basis = load("basis", [128, 1800], bf16, dbasis[:, :])
            xT = [load(f"xT{c}", [DCW[c], B * SC], bf16,
                       dxT[c * 128 : c * 128 + DCW[c], :]) for c in range(3)]
            ahat = load("ahat", [128, 4 * SC], bf16, dahat[:, :])
            root = load("root", [128, 900], bf16, droot[:, :])
            bias = load("bias", [128, 3], f32, dbias[:, :])
            w1a = load("w1a", [128, 1536], bf16, dw1a[:, :])
            w1c = load("w1c", [128, 1536], bf16, dw1c[:, :])
            peR = [pp.tile([128, FPC], bf16, tag=f"peR{mc}", name=f"peR{mc}")
                   for mc in range(4)]
            for mc in range(4):
                nc.sync.dma_start(peR[mc][:, 0:CH], dper[:, mc * FPC : mc * FPC + CH])
            # fp8 GEMM2 weights: one tile, sliced [128, 2, 128] per (s, n)
            w28t = load("w28t", [128, 2048], fp8, dw2f8[:, :])
            w28 = [[w28t[:, (s * 4 + n) * 256 : (s * 4 + n) * 256 + 256]
                    .rearrange("p (two m) -> p two m", two=2)
                    for n in range(4)] for s in range(2)]
            w2b = load("w2b", [128, 2 * M], bf16, dw2b[:, :])
            for mc in range(4):
                nc.sync.dma_start(peR[mc][:, CH:FPC],
                                  dper[:, mc * FPC + CH : (mc + 1) * FPC])
            wp = load("wp", [128, 4], bf16, dwp[:, :])

            hT = [[pp.tile([DCW[ec], SC], bf16, tag=f"hT{b}{ec}", name=f"hT{b}{ec}")
                   for ec in range(3)] for b in range(B)]
            uT = [[pp.tile([128, S], bf16, tag=f"uT{b}{mc}", name=f"uT{b}{mc}")
                   for mc in range(4)] for b in range(B)]
            vT = [[pp.tile([128, IPC], f32, tag=f"vT{b}{mc}", name=f"vT{b}{mc}")
                   for mc in range(4)] for b in range(B)]

            # bulk-TT parts: t[b][mc] = u[b][mc] (broadcast over i) + peR[mc],
            # emitted in 8 half-row pieces so DVE is never blocked for long.
            tbs = {}
            HROWS = [(0, 13), (13, IPC)]

            def emit_tt_part(b, part):
                if b not in tbs:
                    tbs[b] = tp.tile([128, 4, FPC], bf16, tag="tb", name=f"tb{b}")
                tb = tbs[b]
                half, mc = divmod(part, 4)
                r0, r1 = HROWS[half]
                rows = r1 - r0
                nc.vector.tensor_add(
                    tb[:, mc, r0 * S : r1 * S].rearrange("p (r j) -> p r j", r=rows),
                    uT[b][mc][:, :].unsqueeze(1).broadcast_to([128, rows, S]),
                    peR[mc][:, r0 * S : r1 * S].rearrange("p (r j) -> p r j", r=rows))

            # ---------------- stage A: RGCN h, then u/v ----------
            with tc.tile_pool(name="psA", bufs=2, space="PSUM") as psA:
                t1 = [[[pp.tile([JCW[jc], D], bf16, tag=f"t1_{b}{k}{jc}",
                                name=f"t1_{b}{k}{jc}")
                        for jc in range(2)] for k in range(2)] for b in range(B)]

                def emit_t1(b):
                    for k in range(2):
                        for jc in range(2):
                            t1ps = psA.tile([JCW[jc], D], f32, tag="mps", name="t1ps")
                            for dc in range(3):
                                nc.tensor.matmul(
                                    t1ps[:, :],
                                    xT[dc][:, b * SC + jc * 128 : b * SC + jc * 128 + JCW[jc]],
                                    basis[0 : DCW[dc], (k * 3 + dc) * D : (k * 3 + dc) * D + D],
                                    start=(dc == 0), stop=(dc == 2),
                                )
                            nc.scalar.activation(t1[b][k][jc][:, :], t1ps[:, :], AF.Copy)

                def emit_h(b):
                    for ec in range(3):
                        hps = psA.tile([DCW[ec], SC], f32, tag="hps", name="hps")
                        first = True
                        for k in range(2):
                            for jc in range(2):
                                nc.tensor.matmul(
                                    hps[:, :],
                                    t1[b][k][jc][:, ec * 128 : ec * 128 + DCW[ec]],
                                    ahat[0 : JCW[jc], (k * 2 + jc) * SC : (k * 2 + jc + 1) * SC],
                                    start=first, stop=False)
                                first = False
                        for dc in range(3):
                            nc.tensor.matmul(
                                hps[:, :],
                                root[0 : DCW[dc], dc * D + ec * 128 : dc * D + ec * 128 + DCW[ec]],
                                xT[dc][:, b * SC : (b + 1) * SC],
                                start=False, stop=(dc == 2))
                        nc.scalar.activation(hT[b][ec][:, :], hps[:, :], AF.Identity,
                                             bias=bias[0 : DCW[ec], ec : ec + 1])

                def emit_uv(b):
                    for mc in range(4):
                        ups = psA.tile([128, SC], f32, tag="uvps", name="ups")
                        for ec in range(3):
                            nc.tensor.matmul(
                                ups[:, 0:S],
                                w1a[0 : DCW[ec], ec * M + mc * 128 : ec * M + mc * 128 + 128],
                                hT[b][ec][:, 0:S], start=(ec == 0), stop=False)
                        for ec in range(3):
                            nc.tensor.matmul(
                                ups[:, S:SC],
                                w1c[0 : DCW[ec], ec * M + mc * 128 : ec * M + mc * 128 + 128],
                                hT[b][ec][:, S:SC], start=(ec == 0), stop=(ec == 2))
                        nc.scalar.activation(uT[b][mc][:, :], ups[:, 0:S], AF.Copy)
                        nc.scalar.activation(vT[b][mc][:, :], ups[:, S:SC], AF.Copy)

                # b=0 chain first so its bulk-TT (DVE) overlaps the rest of
                # stage A on the PE.
                emit_t1(0)
                emit_h(0)
                emit_uv(0)
                for part in range(8):
                    emit_tt_part(0, part)
                for b in range(1, B):
                    emit_t1(b)
                for b in range(1, B):
                    emit_h(b)
                for b in range(1, B):
                    emit_uv(b)

            # ---------------- stage B: the pairwise MLP ------------------
            with (
                tc.tile_pool(name="ps2", bufs=5, space="PSUM") as ps2,
                tc.tile_pool(name="pp3", bufs=2, space="PSUM") as pp3,
            ):
                def emit_g3(prev):
                    # GEMM3 for the previous unit: h2 chunks stationary,
                    # wp moving -> out [pairs<=100, 1] into the b's pout col.
                    p_u, p_nch, p_pout, p_rh2 = prev
                    for pc in range(p_nch):
                        col = p_u * 4 + pc
                        for mc in range(4):
                            nc.tensor.matmul(
                                p_pout[0:100, col : col + 1],
                                p_rh2[mc][:, pc * 100 : pc * 100 + 100],
                                wp[:, mc : mc + 1],
                                start=(mc == 0), stop=(mc == 3))

                prev = None
                for b in range(B):
                    tb = tbs[b]
                    pout = pp3.tile([128, NCOL], f32, tag="pout", name="pout")
                    sig = psig.tile([128, NCOL], f32, tag="sigb", name="sigb")
                    for u in range(NU):
                        nil = 2 if u < NU - 1 else 1
                        ncols = nil * S
                        # rh1: fp8 for K-chunks 0,1 (Pool), bf16 for 2,3 (DVE)
                        r8 = pwork.tile([128, 2, 400], fp8, tag="r8", name="r8")
                        rb = [pwork.tile([128, 400], bf16, tag=f"rb{j}", name=f"rb{j}")
                              for j in range(2)]
                        for mc in range(4):
                            for h in range(nil):
                                src = tb[:, mc, u * 400 + h * S : u * 400 + h * S + S]
                                if mc < 2:
                                    nc.gpsimd.tensor_scalar(
                                        out=r8[:, mc, h * S : h * S + S],
                                        in0=src,
                                        scalar1=vT[b][mc][:, 2 * u + h : 2 * u + h + 1],
                                        scalar2=0.0, op0=OP.add, op1=OP.max)
                                else:
                                    nc.vector.tensor_scalar(
                                        out=rb[mc - 2][:, h * S : h * S + S],
                                        in0=src,
                                        scalar1=vT[b][mc][:, 2 * u + h : 2 * u + h + 1],
                                        scalar2=0.0, op0=OP.add, op1=OP.max)
                        # GEMM2 + relu2
                        rh2 = [pwork.tile([128, 400], bf16, tag=f"rh2_{n}", name=f"rh2_{n}")
                               for n in range(4)]
                        for n in range(4):
                            ops = ps2.tile([128, 400], f32, tag="ops", name="ops")
                            for s in range(2):
                                nc.tensor.matmul(
                                    ops[:, :ncols],
                                    w28[s][n][:, :, :],
                                    r8[:, :, :ncols],
                                    start=(s == 0), stop=False,
                                    perf_mode=PM.DoubleRow)
                            for j in range(2):
                                nc.tensor.matmul(
                                    ops[:, :ncols],
                                    w2b[:, j * M + n * 128 : j * M + n * 128 + 128],
                                    rb[j][:, :ncols],
                                    start=False, stop=(j == 1))
                            # relu2 split: DVE has slack in later batches
                            # (no more bulk-TT), Act is the constraint there.
                            on_dve = ((b < 2 and n == 3 and u % 2 == 0)
                                      or (b == 2 and n == 3)
                                      or (b == 3 and n >= 2))
                            if on_dve:
                                nc.vector.tensor_scalar(
                                    out=rh2[n][:, :ncols], in0=ops[:, :ncols],
                                    scalar1=0.0, scalar2=None, op0=OP.max)
                            else:
                                nc.scalar.activation(rh2[n][:, :ncols], ops[:, :ncols], AF.Relu)
                        # GEMM3 of the PREVIOUS unit (hides relu2 latency)
                        if prev is not None:
                            emit_g3(prev)
                        prev = (u, 4 if nil == 2 else 2, pout, rh2)
                        # spread the next batch's bulk-TT parts over units 2..9
                        if 2 <= u <= 9 and b + 1 < B:
                            emit_tt_part(b + 1, u - 2)
                        # first 6 units' output cols are final once g3(5) ran
                        if u == 7:
                            nc.scalar.activation(sig[0:100, 0:24], pout[0:100, 0:24],
                                                 AF.Sigmoid)
                            nc.sync.dma_start(dout[b * 100 : b * 100 + 100, 0:24],
                                              sig[0:100, 0:24])
                    # cols 24:48 are final after g3(11) (emitted in the u=12
                    # iteration above); only unit 12's 2 cols remain.
                    nc.scalar.activation(sig[0:100, 24:48], pout[0:100, 24:48],
                                         AF.Sigmoid)
                    nc.sync.dma_start(dout[b * 100 : b * 100 + 100, 24:48],
                                      sig[0:100, 24:48])
                    emit_g3(prev)
                    prev = None
                    nc.scalar.activation(sig[0:100, 48:NCOL], pout[0:100, 48:NCOL],
                                         AF.Sigmoid)
                    nc.sync.dma_start(dout[b * 100 : b * 100 + 100, 48:NCOL],
                                      sig[0:100, 48:NCOL])

    nc.compile()
    return nc


def _host_prep(x, pe_k, pe_v, comp, basis, root, rgcn_bias, W1):
    import ml_dtypes

    bf = ml_dtypes.bfloat16
    f8 = ml_dtypes.float8_e4m3

    ra = _rel_adj(S) % NREL
    onehot = (ra[None, :, :] == np.arange(NREL)[:, None, None]).astype(np.float64)
    deg = onehot.sum(1)
    inv = np.where(deg > 0, 1.0 / np.maximum(deg, 1.0), 0.0)
    anorm = onehot * inv[:, None, :]
    ahat_full = np.einsum("rk,rij->kij", np.asarray(comp, np.float64), anorm)
    ahat_full = ahat_full.astype(np.float32)  # [2, S, S]
    pos = np.clip(np.arange(S)[:, None] - np.arange(S)[None, :] + 1, 0, MAXL)

    x = np.asarray(x, np.float32)
    W1 = np.asarray(W1, np.float32)
    W1a, W1b = W1[:D], W1[D : D + P]
    W1c, W1d = W1[D + P : 2 * D + P], W1[2 * D + P :]
    ttab = (np.asarray(pe_k, np.float64) @ W1b.astype(np.float64)
            + np.asarray(pe_v, np.float64) @ W1d.astype(np.float64)).astype(np.float32)
    ttab_b = ttab.astype(bf).astype(np.float32)  # [11, 512] as the device sees it

    W2 = np.asarray(W2_GLOBAL, np.float32)
    # fp8 half (K rows 0..255): hi + lo residual pair
    W2hi = W2[:256].astype(f8)
    W2lo = (W2[:256] - W2hi.astype(np.float32)).astype(f8)
    w2f8 = np.zeros((128, 2048), f8)
    for s, Wq in enumerate((W2hi, W2lo)):
        for n in range(4):
            for i in range(2):
                w2f8[:, (s * 4 + n) * 256 + i * 128 : (s * 4 + n) * 256 + i * 128 + 128] = \
                    Wq[i * 128 : i * 128 + 128, n * 128 : n * 128 + 128]
    # bf16 half (K rows 256..511)
    w2b = np.zeros((128, 2 * M), np.float32)
    for j in range(2):
        w2b[:, j * M : (j + 1) * M] = W2[(2 + j) * 128 : (3 + j) * 128, :]

    com = {
        "basis": np.concatenate(
            [_pack_k(np.asarray(basis[k], np.float32)) for k in range(2)], axis=1
        ).astype(bf),
        "root": _pack_k(np.asarray(root, np.float32)).astype(bf),
        "w1a": _pack_k(W1a).astype(bf),
        "w1c": _pack_k(W1c).astype(bf),
        "w2f8": w2f8,
        "w2b": w2b.astype(bf),
        "wp": np.ascontiguousarray(np.asarray(WP_GLOBAL, np.float32)[:, 0]
                                   .reshape(4, 128).T).astype(bf),
    }
    bias_p = np.zeros((128, 3), np.float32)
    rb = np.asarray(rgcn_bias, np.float32)
    for c in range(3):
        r = min(128, D - c * 128)
        bias_p[:r, c] = rb[c * 128 : c * 128 + r]
    com["bias"] = bias_p

    SC = S + IPC
    xt_all = x.transpose(2, 0, 1)  # [D, B, S]
    per_core = []
    for c in range(NCORES):
        i0 = c * IPC
        m = dict(com)
        xtc = np.empty((D, B * SC), np.float32)
        for b in range(B):
            xtc[:, b * SC : b * SC + S] = xt_all[:, b, :]
            xtc[:, b * SC + S : (b + 1) * SC] = xt_all[:, b, i0 : i0 + IPC]
        m["xT"] = xtc.astype(bf)
        ah = np.zeros((128, 4 * SC), np.float32)
        for k in range(2):
            for jc in range(2):
                r = 128 if jc == 0 else 72
                base = (k * 2 + jc) * SC
                ah[:r, base : base + S] = ahat_full[k, jc * 128 : jc * 128 + r, :]
                ah[:r, base + S : base + SC] = ahat_full[k, jc * 128 : jc * 128 + r, i0 : i0 + IPC]
        m["ahat"] = ah.astype(bf)
        # peR: [128, mc*FPC + pair] = T[pos(pair), mc*128+p]
        pe_pairs = ttab_b[pos[i0 : i0 + IPC, :].reshape(-1)]  # [FPC, 512]
        m["per"] = np.ascontiguousarray(
            pe_pairs.T.reshape(4, 128, FPC).reshape(512, FPC)
            .reshape(4, 128, FPC).transpose(1, 0, 2).reshape(128, 4 * FPC)).astype(bf)
        per_core.append(m)
    return per_core


W2_GLOBAL = None
WP_GLOBAL = None


def kernel(x, mask, pe_k, pe_v, comp, basis, root, rgcn_bias, W1, W2, Wp,
           _want_results=False, _trace=False):
    global W2_GLOBAL, WP_GLOBAL
    W2_GLOBAL, WP_GLOBAL = W2, Wp

    from concourse.bass_utils import run_bass_kernel_spmd

    if "nc" not in _prog_cache:
        _prog_cache["nc"] = _build_program()
    nc = _prog_cache["nc"]

    in_maps = _host_prep(x, pe_k, pe_v, comp, basis, root, rgcn_bias, W1)
    res = run_bass_kernel_spmd(nc, in_maps, core_ids=list(range(NCORES)),
                               trace=_trace)

    out = np.zeros((B, S, S), np.float32)
    for c in range(NCORES):
        i0 = c * IPC
        arr = np.asarray(res.results[c]["out"], np.float32).reshape(B, 100, NCOL)
        out[:, i0 : i0 + IPC, :] = arr.transpose(0, 2, 1).reshape(B, IPC, S)
    out *= np.asarray(mask, np.float32)
    if _want_results:
        return out, res
    return out


# revision 22
# speedup vs baseline: 2.0633x; 1.0057x over previous
"""Trainium2 Bass kernel for nn_CausePredictor (RGCN + pairwise MLP).

Sharding: data-parallel over the pairwise row index i (dim 1 of the
[B,S,S] output): 8 cores x 25 rows, replicated over B=4.  All per-core
differences are encoded as input DATA (column slices / gathered pe
tables), so one SPMD program serves all cores.

Math (matching reference.py):
  h   = sum_k Ahat_k.T @ (x[b] @ basis_k) + x[b] @ root + bias
  u   = h @ W1a   (j term),  v = h @ W1c  (i term)
  T   = pe_k @ W1b + pe_v @ W1d          # [11, 512], host precomputed
  h1[b,i,j,:] = u[b,j] + v[b,i] + T[pos(i,j)]
  out = sigmoid(Wp . relu(relu(h1) @ W2)) * mask

Performance structure (per TimelineSim cost model):
  - stage A (RGCN + u/v) runs in bf16 (fp32 matmuls are 4x slower).
  - peR = T[pos] pair-expansion is precomputed on HOST and DMA'd in.
  - t = u + peR is ONE broadcast tensor_add per (b, mc) on DVE.
  - rh1 = relu(t + v_i): K-chunks 0,1 quantize to fp8 (on Pool),
    chunks 2,3 stay bf16 (on DVE).
  - GEMM2: fp8 chunks use DoubleRow matmuls (0.5 cyc/row) with W2
    split into hi+lo fp8 pair (weight quantization ~exact); bf16
    chunks use normal matmuls.  3 cyc/row total vs 4 for pure bf16.
  - relu2 PSUM->SBUF copies split 3:1 between Activation and Pool.
  - GEMM3 (the Wp dot) runs with h2 chunks STATIONARY and wp moving:
    out is [pairs<=100, 1], ~1 PE row per matmul instead of 400.
"""

import sys

sys.path.insert(0, "/opt/trn_rl_repo")

import numpy as np

B, S, D, M, P = 4, 200, 300, 512, 100
NREL, MAXL = 9, 10
NCORES = 8
IPC = S // NCORES  # 25 rows of i per core
NU = IPC // 2 + 1  # 13 units per b: 12x 2-row + 1x 1-row
FPC = IPC * S  # 5000 pairs per (b, core)
NCOL = 50  # output columns per b: each col = 100 pairs

_prog_cache = {}


def _rel_adj(s):
    ra = np.arange(s)[None, :] - np.arange(s)[:, None]
    for i in range(s):
        ra[i, i + 1 :] = 1
        num = 1
        for o in range(i - 1, -1, -2):
            ra[i, o] = -num
            if o - 1 >= 0:
                ra[i, o - 1] = -num
            num += 1
        ra[i, :i] = np.maximum(ra[i, :i], -8)  # -(WINDOW+1), WINDOW=7
    return ra


def _pack_k(w, width=None):
    """[K, N] -> [128, ceil(K/128)*N], K chunked onto partitions, zero pad."""
    k, n = w.shape
    nch = (k + 127) // 128
    out = np.zeros((128, nch * n), np.float32)
    for c in range(nch):
        r = min(128, k - c * 128)
        out[:r, c * n : c * n + n] = w[c * 128 : c * 128 + r]
    return out


def _build_program():
    import ml_dtypes  # noqa: F401
    import concourse.tile as tile
    from concourse import bacc, mybir

    f32 = mybir.dt.float32
    bf16 = mybir.dt.bfloat16
    fp8 = mybir.dt.float8e4
    AF = mybir.ActivationFunctionType
    OP = mybir.AluOpType
    PM = mybir.MatmulPerfMode

    nc = bacc.Bacc()

    SC = S + IPC  # 225: concat of all-j columns and the core's i-slice
    dxT = nc.declare_dram_parameter("xT", [D, B * SC], bf16, isOutput=False)
    dahat = nc.declare_dram_parameter("ahat", [128, 4 * SC], bf16, isOutput=False)
    dbasis = nc.declare_dram_parameter("basis", [128, 1800], bf16, isOutput=False)
    droot = nc.declare_dram_parameter("root", [128, 900], bf16, isOutput=False)
    dbias = nc.declare_dram_parameter("bias", [128, 3], f32, isOutput=False)
    dw1a = nc.declare_dram_parameter("w1a", [128, 1536], bf16, isOutput=False)
    dw1c = nc.declare_dram_parameter("w1c", [128, 1536], bf16, isOutput=False)
    dw2f8 = nc.declare_dram_parameter("w2f8", [128, 2048], fp8, isOutput=False)
    dw2b = nc.declare_dram_parameter("w2b", [128, 2 * M], bf16, isOutput=False)
    dwp = nc.declare_dram_parameter("wp", [128, 4], bf16, isOutput=False)
    dper = nc.declare_dram_parameter("per", [128, 4 * FPC], bf16, isOutput=False)
    dout = nc.declare_dram_parameter("out", [B * 100, NCOL], f32, isOutput=True)

    DCW = [128, 128, 44]  # D=300 chunks
    JCW = [128, 72]  # S=200 chunks

    with tile.TileContext(nc) as tc:
        with (
            tc.tile_pool(name="persist", bufs=1) as pp,
            tc.tile_pool(name="tpool", bufs=2) as tp,
            tc.tile_pool(name="work", bufs=3) as pwork,
            tc.tile_pool(name="sigp", bufs=2) as psig,
        ):
            def load(name, shape, dt, src):
                t = pp.tile(shape, dt, tag=name, name=name)
                if len(shape) == 3:
                    nc.sync.dma_start(t[:, :, :], src)
                else:
                    nc.sync.dma_start(t[:, :], src)
                return t

            # DMA order = deadline order.  The serial DMA stream (~23us for
            # 8MB) is a startup critical path: stage-A weights first, then
            # the first-half rows of peR (consumed by the early units),
            # then stage-B weights, then peR's second halves (not needed
            # until ~unit 7 of b=0).
            CH = 13 * S  # peR column split matching the bulk-TT halves
            basis = load("basis", [128, 1800], bf16, dbasis[:, :])
            xT = [load(f"xT{c}", [DCW[c], B * SC], bf16,
                       dxT[c * 128 : c * 128 + DCW[c], :]) for c in range(3)]
            ahat = load("ahat", [128, 4 * SC], bf16, dahat[:, :])
            root = load("root", [128, 900], bf16, droot[:, :])
            bias = load("bias", [128, 3], f32, dbias[:, :])
            w1a = load("w1a", [128, 1536], bf16, dw1a[:, :])
            w1c = load("w1c", [128, 1536], bf16, dw1c[:, :])
            peR = [pp.tile([128, FPC], bf16, tag=f"peR{mc}", name=f"peR{mc}")
                   for mc in range(4)]
            for mc in range(4):
                nc.sync.dma_start(peR[mc][:, 0:CH], dper[:, mc * FPC : mc * FPC + CH])
            # fp8 GEMM2 weights: one tile, sliced [128, 2, 128] per (s, n)
            w28t = load("w28t", [128, 2048], fp8, dw2f8[:, :])
            w28 = [[w28t[:, (s * 4 + n) * 256 : (s * 4 + n) * 256 + 256]
                    .rearrange("p (two m) -> p two m", two=2)
                    for n in range(4)] for s in range(2)]
            w2b = load("w2b", [128, 2 * M], bf16, dw2b[:, :])
            for mc in range(4):
                nc.sync.dma_start(peR[mc][:, CH:FPC],
                                  dper[:, mc * FPC + CH : (mc + 1) * FPC])
            wp = load("wp", [128, 4], bf16, dwp[:, :])

            hT = [[pp.tile([DCW[ec], SC], bf16, tag=f"hT{b}{ec}", name=f"hT{b}{ec}")
                   for ec in range(3)] for b in range(B)]
            # u (cols 0:S) and v (cols S:SC) in ONE tile per (b, mc) so the
            # PSUM->SBUF copy is a single instruction.
            uvT = [[pp.tile([128, SC], bf16, tag=f"uvT{b}{mc}", name=f"uvT{b}{mc}")
                    for mc in range(4)] for b in range(B)]
            vT = [[pp.tile([128, IPC], f32, tag=f"vT{b}{mc}", name=f"vT{b}{mc}")
                   for mc in range(4)] for b in range(B)]

            # bulk-TT parts: t[b][mc] = u[b][mc] (broadcast over i) + peR[mc],
            # emitted in 8 half-row pieces so DVE is never blocked for long.
            tbs = {}
            HROWS = [(0, 13), (13, IPC)]

            def emit_tt_part(b, part):
                if b not in tbs:
                    tbs[b] = tp.tile([128, 4, FPC], bf16, tag="tb", name=f"tb{b}")
                tb = tbs[b]
                half, mc = divmod(part, 4)
                r0, r1 = HROWS[half]
                rows = r1 - r0
                nc.vector.tensor_add(
                    tb[:, mc, r0 * S : r1 * S].rearrange("p (r j) -> p r j", r=rows),
                    uvT[b][mc][:, 0:S].unsqueeze(1).broadcast_to([128, rows, S]),
                    peR[mc][:, r0 * S : r1 * S].rearrange("p (r j) -> p r j", r=rows))

            # ---------------- stage A: RGCN h, then u/v ----------
            with tc.tile_pool(name="psA", bufs=2, space="PSUM") as psA:
                t1 = [[[pp.tile([JCW[jc], D], bf16, tag=f"t1_{b}{k}{jc}",
                                name=f"t1_{b}{k}{jc}")
                        for jc in range(2)] for k in range(2)] for b in range(B)]

                def emit_t1(b):
                    for k in range(2):
                        for jc in range(2):
                            t1ps = psA.tile([JCW[jc], D], f32, tag="mps", name="t1ps")
                            for dc in range(3):
                                nc.tensor.matmul(
                                    t1ps[:, :],
                                    xT[dc][:, b * SC + jc * 128 : b * SC + jc * 128 + JCW[jc]],
                                    basis[0 : DCW[dc], (k * 3 + dc) * D : (k * 3 + dc) * D + D],
                                    start=(dc == 0), stop=(dc == 2),
                                )
                            if b == 0:
                                nc.vector.tensor_copy(t1[b][k][jc][:, :], t1ps[:, :])
                            else:
                                nc.scalar.activation(t1[b][k][jc][:, :], t1ps[:, :], AF.Copy)

                def emit_h(b):
                    for ec in range(3):
                        hps = psA.tile([DCW[ec], SC], f32, tag="hps", name="hps")
                        first = True
                        for k in range(2):
                            for jc in range(2):
                                nc.tensor.matmul(
                                    hps[:, :],
                                    t1[b][k][jc][:, ec * 128 : ec * 128 + DCW[ec]],
                                    ahat[0 : JCW[jc], (k * 2 + jc) * SC : (k * 2 + jc + 1) * SC],
                                    start=first, stop=False)
                                first = False
                        for dc in range(3):
                            nc.tensor.matmul(
                                hps[:, :],
                                root[0 : DCW[dc], dc * D + ec * 128 : dc * D + ec * 128 + DCW[ec]],
                                xT[dc][:, b * SC : (b + 1) * SC],
                                start=False, stop=(dc == 2))
                        if b == 0:
                            nc.vector.tensor_scalar(
                                out=hT[b][ec][:, :], in0=hps[:, :],
                                scalar1=bias[0 : DCW[ec], ec : ec + 1],
                                scalar2=None, op0=OP.add)
                        else:
                            nc.scalar.activation(hT[b][ec][:, :], hps[:, :], AF.Identity,
                                                 bias=bias[0 : DCW[ec], ec : ec + 1])

                def emit_uv(b):
                    # b=0's copies stay on Act (its critical path); later
                    # batches copy on DVE so Act's copy chain never gates
                    # the PE through the stage-A tail.
                    for mc in range(4):
                        ups = psA.tile([128, SC], f32, tag="uvps", name="ups")
                        for ec in range(3):
                            nc.tensor.matmul(
                                ups[:, 0:S],
                                w1a[0 : DCW[ec], ec * M + mc * 128 : ec * M + mc * 128 + 128],
                                hT[b][ec][:, 0:S], start=(ec == 0), stop=False)
                        for ec in range(3):
                            nc.tensor.matmul(
                                ups[:, S:SC],
                                w1c[0 : DCW[ec], ec * M + mc * 128 : ec * M + mc * 128 + 128],
                                hT[b][ec][:, S:SC], start=(ec == 0), stop=(ec == 2))
                        if b == 0:
                            nc.vector.tensor_copy(uvT[b][mc][:, :], ups[:, :])
                            nc.vector.tensor_copy(vT[b][mc][:, :], ups[:, S:SC])
                        else:
                            nc.scalar.activation(uvT[b][mc][:, :], ups[:, :], AF.Copy)
                            nc.scalar.activation(vT[b][mc][:, :], ups[:, S:SC], AF.Copy)

                # b=0 chain first so its bulk-TT (DVE) overlaps the rest of
                # stage A on the PE.
                emit_t1(0)
                emit_h(0)
                emit_uv(0)
                for part in range(4):  # h0 rows; h1 parts go inside stage B
                    emit_tt_part(0, part)
                for b in range(1, B):
                    emit_t1(b)
                for b in range(1, B):
                    emit_h(b)
                for b in range(1, B):
                    emit_uv(b)

            # ---------------- stage B: the pairwise MLP ------------------
            with (
                tc.tile_pool(name="ps2", bufs=5, space="PSUM") as ps2,
                tc.tile_pool(name="pp3", bufs=2, space="PSUM") as pp3,
            ):
                def emit_g3(prev):
                    # GEMM3 for the previous unit: h2 chunks stationary,
                    # wp moving -> out [pairs<=100, 1] into the b's pout col.
                    p_u, p_nch, p_pout, p_rh2 = prev
                    for pc in range(p_nch):
                        col = p_u * 4 + pc
                        for mc in range(4):
                            nc.tensor.matmul(
                                p_pout[0:100, col : col + 1],
                                p_rh2[mc][:, pc * 100 : pc * 100 + 100],
                                wp[:, mc : mc + 1],
                                start=(mc == 0), stop=(mc == 3))

                prev = None
                for b in range(B):
                    tb = tbs[b]
                    pout = pp3.tile([128, NCOL], f32, tag="pout", name="pout")
                    sig = psig.tile([128, NCOL], f32, tag="sigb", name="sigb")
                    for u in range(NU):
                        nil = 2 if u < NU - 1 else 1
                        ncols = nil * S
                        # rh1: fp8 for K-chunks 0,1 (Pool), bf16 for 2,3 (DVE)
                        r8 = pwork.tile([128, 2, 400], fp8, tag="r8", name="r8")
                        rb = [pwork.tile([128, 400], bf16, tag=f"rb{j}", name=f"rb{j}")
                              for j in range(2)]
                        for mc in range(4):
                            for h in range(nil):
                                src = tb[:, mc, u * 400 + h * S : u * 400 + h * S + S]
                                vsc = vT[b][mc][:, 2 * u + h : 2 * u + h + 1]
                                if mc < 2:
                                    nc.gpsimd.tensor_scalar(
                                        out=r8[:, mc, h * S : h * S + S],
                                        in0=src, scalar1=vsc,
                                        scalar2=0.0, op0=OP.add, op1=OP.max)
                                else:
                                    nc.vector.tensor_scalar(
                                        out=rb[mc - 2][:, h * S : h * S + S],
                                        in0=src, scalar1=vsc,
                                        scalar2=0.0, op0=OP.add, op1=OP.max)
                        # GEMM2 + relu2
                        rh2 = [pwork.tile([128, 400], bf16, tag=f"rh2_{n}", name=f"rh2_{n}")
                               for n in range(4)]
                        for n in range(4):
                            ops = ps2.tile([128, 400], f32, tag="ops", name="ops")
                            for s in range(2):
                                nc.tensor.matmul(
                                    ops[:, :ncols],
                                    w28[s][n][:, :, :],
                                    r8[:, :, :ncols],
                                    start=(s == 0), stop=False,
                                    perf_mode=PM.DoubleRow)
                            for j in range(2):
                                nc.tensor.matmul(
                                    ops[:, :ncols],
                                    w2b[:, j * M + n * 128 : j * M + n * 128 + 128],
                                    rb[j][:, :ncols],
                                    start=False, stop=(j == 1))
                            # relu2 split: DVE has slack in later batches
                            # (no more bulk-TT), Act is the constraint there.
                            on_dve = ((b < 2 and n == 3 and u % 2 == 0)
                                      or (b == 2 and n == 3)
                                      or (b == 3 and n >= 2))
                            if on_dve:
                                nc.vector.tensor_scalar(
                                    out=rh2[n][:, :ncols], in0=ops[:, :ncols],
                                    scalar1=0.0, scalar2=None, op0=OP.max)
                            else:
                                nc.scalar.activation(rh2[n][:, :ncols], ops[:, :ncols], AF.Relu)
                        # GEMM3 of the PREVIOUS unit (hides relu2 latency)
                        if prev is not None:
                            emit_g3(prev)
                        prev = (u, 4 if nil == 2 else 2, pout, rh2)
                        # b=0's TT second-half parts wait for the late peR
                        # columns; emit them where the DMA has landed.
                        if b == 0 and 2 <= u <= 5:
                            emit_tt_part(0, 4 + (u - 2))
                        # spread the next batch's bulk-TT parts over units 2..9
                        if 2 <= u <= 9 and b + 1 < B:
                            emit_tt_part(b + 1, u - 2)
                        # first 6 units' output cols are final once g3(5) ran
                        if u == 7:
                            nc.scalar.activation(sig[0:100, 0:24], pout[0:100, 0:24],
                                                 AF.Sigmoid)
                            nc.sync.dma_start(dout[b * 100 : b * 100 + 100, 0:24],
                                              sig[0:100, 0:24])
                    # cols 24:48 are final after g3(11) (emitted in the u=12
                    # iteration above); only unit 12's 2 cols remain.
                    nc.scalar.activation(sig[0:100, 24:48], pout[0:100, 24:48],
                                         AF.Sigmoid)
                    nc.sync.dma_start(dout[b * 100 : b * 100 + 100, 24:48],
                                      sig[0:100, 24:48])
                    emit_g3(prev)
                    prev = None
                    nc.scalar.activation(sig[0:100, 48:NCOL], pout[0:100, 48:NCOL],
                                         AF.Sigmoid)
                    nc.sync.dma_start(dout[b * 100 : b * 100 + 100, 48:NCOL],
                                      sig[0:100, 48:NCOL])

    nc.compile()
    return nc


def _host_prep(x, pe_k, pe_v, comp, basis, root, rgcn_bias, W1):
    import ml_dtypes

    bf = ml_dtypes.bfloat16
    f8 = ml_dtypes.float8_e4m3

    ra = _rel_adj(S) % NREL
    onehot = (ra[None, :, :] == np.arange(NREL)[:, None, None]).astype(np.float64)
    deg = onehot.sum(1)
    inv = np.where(deg > 0, 1.0 / np.maximum(deg, 1.0), 0.0)
    anorm = onehot * inv[:, None, :]
    ahat_full = np.einsum("rk,rij->kij", np.asarray(comp, np.float64), anorm)
    ahat_full = ahat_full.astype(np.float32)  # [2, S, S]
    pos = np.clip(np.arange(S)[:, None] - np.arange(S)[None, :] + 1, 0, MAXL)

    x = np.asarray(x, np.float32)
    W1 = np.asarray(W1, np.float32)
    W1a, W1b = W1[:D], W1[D : D + P]
    W1c, W1d = W1[D + P : 2 * D + P], W1[2 * D + P :]
    ttab = (np.asarray(pe_k, np.float64) @ W1b.astype(np.float64)
            + np.asarray(pe_v, np.float64) @ W1d.astype(np.float64)).astype(np.float32)
    ttab_b = ttab.astype(bf).astype(np.float32)  # [11, 512] as the device sees it

    W2 = np.asarray(W2_GLOBAL, np.float32)
    # fp8 half (K rows 0..255): hi + lo residual pair
    W2hi = W2[:256].astype(f8)
    W2lo = (W2[:256] - W2hi.astype(np.float32)).astype(f8)
    w2f8 = np.zeros((128, 2048), f8)
    for s, Wq in enumerate((W2hi, W2lo)):
        for n in range(4):
            for i in range(2):
                w2f8[:, (s * 4 + n) * 256 + i * 128 : (s * 4 + n) * 256 + i * 128 + 128] = \
                    Wq[i * 128 : i * 128 + 128, n * 128 : n * 128 + 128]
    # bf16 half (K rows 256..511)
    w2b = np.zeros((128, 2 * M), np.float32)
    for j in range(2):
        w2b[:, j * M : (j + 1) * M] = W2[(2 + j) * 128 : (3 + j) * 128, :]

    com = {
        "basis": np.concatenate(
            [_pack_k(np.asarray(basis[k], np.float32)) for k in range(2)], axis=1
        ).astype(bf),
        "root": _pack_k(np.asarray(root, np.float32)).astype(bf),
        "w1a": _pack_k(W1a).astype(bf),
        "w1c": _pack_k(W1c).astype(bf),
        "w2f8": w2f8,
        "w2b": w2b.astype(bf),
        "wp": np.ascontiguousarray(np.asarray(WP_GLOBAL, np.float32)[:, 0]
                                   .reshape(4, 128).T).astype(bf),
    }
    bias_p = np.zeros((128, 3), np.float32)
    rb = np.asarray(rgcn_bias, np.float32)
    for c in range(3):
        r = min(128, D - c * 128)
        bias_p[:r, c] = rb[c * 128 : c * 128 + r]
    com["bias"] = bias_p

    SC = S + IPC
    xt_all = x.transpose(2, 0, 1)  # [D, B, S]
    per_core = []
    for c in range(NCORES):
        i0 = c * IPC
        m = dict(com)
        xtc = np.empty((D, B * SC), np.float32)
        for b in range(B):
            xtc[:, b * SC : b * SC + S] = xt_all[:, b, :]
            xtc[:, b * SC + S : (b + 1) * SC] = xt_all[:, b, i0 : i0 + IPC]
        m["xT"] = xtc.astype(bf)
        ah = np.zeros((128, 4 * SC), np.float32)
        for k in range(2):
            for jc in range(2):
                r = 128 if jc == 0 else 72
                base = (k * 2 + jc) * SC
                ah[:r, base : base + S] = ahat_full[k, jc * 128 : jc * 128 + r, :]
                ah[:r, base + S : base + SC] = ahat_full[k, jc * 128 : jc * 128 + r, i0 : i0 + IPC]
        m["ahat"] = ah.astype(bf)
        # peR: [128, mc*FPC + pair] = T[pos(pair), mc*128+p]
        pe_pairs = ttab_b[pos[i0 : i0 + IPC, :].reshape(-1)]  # [FPC, 512]
        m["per"] = np.ascontiguousarray(
            pe_pairs.T.reshape(4, 128, FPC).reshape(512, FPC)
            .reshape(4, 128, FPC).transpose(1, 0, 2).reshape(128, 4 * FPC)).astype(bf)
        per_core.append(m)
    return per_core


W2_GLOBAL = None
WP_GLOBAL = None


def kernel(x, mask, pe_k, pe_v, comp, basis, root, rgcn_bias, W1, W2, Wp,
           _want_results=False, _trace=False):
    global W2_GLOBAL, WP_GLOBAL
    W2_GLOBAL, WP_GLOBAL = W2, Wp

    from concourse.bass_utils import run_bass_kernel_spmd

    if "nc" not in _prog_cache:
        _prog_cache["nc"] = _build_program()
    nc = _prog_cache["nc"]

    in_maps = _host_prep(x, pe_k, pe_v, comp, basis, root, rgcn_bias, W1)
    res = run_bass_kernel_spmd(nc, in_maps, core_ids=list(range(NCORES)),
                               trace=_trace)

    out = np.zeros((B, S, S), np.float32)
    for c in range(NCORES):
        i0 = c * IPC
        arr = np.asarray(res.results[c]["out"], np.float32).reshape(B, 100, NCOL)
        out[:, i0 : i0 + IPC, :] = arr.transpose(0, 2, 1).reshape(B, IPC, S)
    out *= np.asarray(mask, np.float32)
    if _want_results:
        return out, res
    return out
